# revision 7
# baseline (speedup 1.0000x reference)
"""GCNII backbone Bass/Trainium2 kernel — 8-core SPMD, v2.

Sharding: nodes row-partitioned across 8 cores (12500/core, padded to 12544
= 98 tiles of 128).  Edges live on the core that owns their *destination*
node.  Host-side index-only preprocessing builds a window-major,
destination-sorted, capacity-padded edge stream per core; the device does
everything float.

v2 structure (vs v1):
  * segment matrix B is built ON DEVICE per 128x128 chunk with one DVE
    dual-op tensor_scalar (is_equal by col, mult by enorm) from two small
    resident vectors — no 70MB/layer HBM stream of B.
  * f_full is laid out tile-major-interleaved and split into 4 window
    tensors f_full[w] (<=32768 rows each, int16-indexable).  Each window is
    AllGathered separately (Shared output) as soon as every core stored its
    slice piece, and the next layer's window-w gathers depend only on
    sub-AG w -> collectives pipeline behind compute.
  * window-major schedule: for w: gather spans of 14 dst tiles, chunk
    matmuls accumulate into PSUM per (tile,window), DVE folds into an SBUF
    f32 accumulator H; last window also feeds the dense P matmuls, stats,
    AllReduce, normalize, transpose, store, sub-AGs.
"""

import os
import sys

for _p in ("/opt/trn_rl_repo",):
    if _p not in sys.path:
        sys.path.insert(0, _p)

import math

import ml_dtypes
import numpy as np

import concourse.bacc as bacc
import concourse.bass as bass
import concourse.tile as tile
from concourse import mybir
from concourse.bass_utils import run_bass_kernel_spmd

F32 = mybir.dt.float32
BF16 = mybir.dt.bfloat16
I16 = mybir.dt.int16
AX = mybir.AxisListType
AL = mybir.AluOpType
AF = mybir.ActivationFunctionType

NCORES = 8
D = 128
DIN = 256
L = 4
ALPHA = 0.5
THETA = 1.0
EPS = 1e-5


def _mk_cfg(N, slice_, sp):
    pad = ((slice_ + 127) // 128) * 128
    nt = pad // 128
    # split tiles into 4 windows (gather source ranges; one sub-AG each)
    base = -(-nt // 4)
    wt = []
    t0 = 0
    for w in range(4):
        t1 = min(t0 + base, nt)
        wt.append((t0, t1))
        t0 = t1
    assert wt[-1][1] == nt
    for (a, b) in wt:
        assert (b - a) * 128 * NCORES <= 32768  # int16-indexable window
    # spans: dst tiles per gather call; every src window sweeps ALL dst tiles
    spans = []
    for w in range(4):
        s0 = 0
        while s0 < nt:
            s1 = min(s0 + sp, nt)
            spans.append((w, s0, s1))
            s0 = s1
    return dict(N=N, SLICE=slice_, PAD=pad, NT=nt, WT=wt, SPANS=spans)


def full_cfg(N=100000):
    return _mk_cfg(N, N // NCORES, 14)


def small_cfg():
    return _mk_cfg(8000, 1000, 2)


# ---------------------------------------------------------------- host prep
def preprocess(x, edge_index, lin1_w, lin1_b, w1, w2, norm_w, norm_b, cfg):
    N, SLICE, PAD, NT = cfg["N"], cfg["SLICE"], cfg["PAD"], cfg["NT"]
    WT, SPANS = cfg["WT"], cfg["SPANS"]

    src = np.asarray(edge_index[0], dtype=np.int64)
    dst = np.asarray(edge_index[1], dtype=np.int64)
    sl = np.arange(N, dtype=np.int64)
    srcA = np.concatenate([src, sl])
    dstA = np.concatenate([dst, sl])

    deg = np.bincount(dstA, minlength=N).astype(np.float64)
    dis = 1.0 / np.sqrt(deg)
    en = ((1.0 - ALPHA) * dis[srcA] * dis[dstA]).astype(np.float32)

    # source address in its window tensor: f_full[w][c*TW*128 + (t-t0)*128 + r]
    s_core = srcA // SLICE
    s_loc = srcA % SLICE
    s_tile = s_loc // 128
    s_row = s_loc % 128
    t0s = np.zeros(NT, np.int64)
    wofs = np.zeros(NT, np.int64)  # window id per tile
    for w, (a, b) in enumerate(WT):
        wofs[a:b] = w
        t0s[a:b] = a
    twsize = np.array([b - a for (a, b) in WT], np.int64)
    s_w = wofs[s_tile]
    addr = (s_core * twsize[s_w] + (s_tile - t0s[s_tile])) * 128 + s_row
    assert addr.max() < 32768

    core = dstA // SLICE
    lt = (dstA % SLICE) // 128
    colr = ((dstA % SLICE) % 128).astype(np.float32)

    # per-(core, dst tile, window) counts -> shared static caps (mult of 128)
    blk = (core * NT + lt) * 4 + s_w
    cnt = np.bincount(blk, minlength=NCORES * NT * 4).reshape(NCORES, NT, 4)
    cap = (np.ceil(cnt.max(axis=0) / 128).astype(np.int64)) * 128  # [NT, 4]

    # stream block order: src-window-major, then dst tile
    border = [(t, w) for w in range(4) for t in range(NT)]
    blk_of = {tw: i for i, tw in enumerate(border)}
    blk_len = np.array([cap[t, w] for (t, w) in border], dtype=np.int64)
    blk_start_arr = np.concatenate([[0], np.cumsum(blk_len)])
    S_total = int(blk_start_arr[-1])
    NCH = S_total // 128
    blk_start = {tw: int(blk_start_arr[i]) for i, tw in enumerate(border)}

    # gather calls: one per span (w, s0, s1)
    call_start = [blk_start[(s0, w)] for (w, s0, s1) in SPANS]
    call_len = [int(cap[s0:s1, w].sum()) for (w, s0, s1) in SPANS]
    first_w = {}  # first window with edges, per tile
    for t in range(NT):
        first_w[t] = int(np.nonzero(cap[t, :])[0][0])

    sched = dict(cap=cap, blk_start=blk_start, call_start=call_start,
                 call_len=call_len, S=S_total, NCH=NCH, first_w=first_w)

    # per-core streams
    per_core = []
    bidx_all = np.array([blk_of[(int(t), int(w))] for t, w in zip(lt, s_w)],
                        dtype=np.int64)
    for c in range(NCORES):
        m = core == c
        bi = bidx_all[m]
        order = np.argsort(bi, kind="stable")
        bi_s = bi[order]
        cnts = np.bincount(bi_s, minlength=len(border))
        starts_sorted = np.concatenate([[0], np.cumsum(cnts)])[:-1]
        rank = np.arange(len(bi_s)) - starts_sorted[bi_s]
        pos = blk_start_arr[bi_s] + rank

        idx_s = np.zeros(S_total, np.int64)
        col_s = np.zeros(S_total, np.int64)
        en_s = np.zeros(S_total, np.float32)
        idx_s[pos] = addr[m][order]
        col_s[pos] = colr[m][order].astype(np.int64)
        en_s[pos] = en[m][order]

        # idx packed per gather call: wrap 16 partitions, replicate x8
        idxp = np.zeros((16, S_total // 16), np.int16)
        for a, ln in zip(call_start, call_len):
            if ln == 0:
                continue
            seg = idx_s[a:a + ln].astype(np.int16)
            idxp[:, a // 16:(a + ln) // 16] = seg.reshape(ln // 16, 16).T
        idxp = np.tile(idxp, (NCORES, 1))

        # chunk-major col / enorm vectors for on-device B build
        colp = col_s.reshape(NCH, 128).T.astype(np.float32)
        enp = en_s.reshape(NCH, 128).T.astype(np.float32)
        colp = np.ascontiguousarray(colp)
        enp = np.ascontiguousarray(enp)

        # x slice, transposed+packed on host: xT[j,k,d] = x[row d, 128j+k]
        xs = np.zeros((PAD, DIN), np.float32)
        xs[:SLICE] = np.asarray(x[c * SLICE:(c + 1) * SLICE], np.float32)
        xT = np.ascontiguousarray(
            xs.T.reshape(2, 128, PAD)).astype(ml_dtypes.bfloat16)

        per_core.append(dict(idx=idxp, colv=colp, env=enp, xT=xT))

    # weights
    lw = np.asarray(lin1_w, np.float32)          # [128, 256]
    lin1wT = np.ascontiguousarray(lw.T.reshape(2, 128, 128)).astype(
        ml_dtypes.bfloat16)
    m1 = np.zeros((L, 128, 128), np.float32)
    m2 = np.zeros((L, 128, 128), np.float32)
    eye = np.eye(128, dtype=np.float32)
    for li in range(L):
        beta = float(np.log(THETA / (li + 1) + 1.0))
        m1[li] = (1.0 - beta) * eye + beta * np.asarray(w1[li], np.float32)
        m2[li] = ALPHA * ((1.0 - beta) * eye + beta * np.asarray(w2[li], np.float32))
    iota = np.tile(np.arange(128, dtype=np.float32), (128, 1))
    consts = dict(
        lin1wT=lin1wT,
        lin1b=np.asarray(lin1_b, np.float32).reshape(128, 1),
        m1=m1.astype(ml_dtypes.bfloat16), m2=m2.astype(ml_dtypes.bfloat16),
        nw=np.asarray(norm_w, np.float32).reshape(128, 1),
        nb=np.asarray(norm_b, np.float32).reshape(128, 1),
        identb=np.eye(128, dtype=np.float32).astype(ml_dtypes.bfloat16),
        identf=np.eye(128, dtype=np.float32),
        iota=iota,
    )
    return sched, per_core, consts


# ---------------------------------------------------------------- device IR
def build(cfg, sched, debug=None):
    debug = debug or {}
    n_layers = debug.get("n_layers", L)
    no_ar = debug.get("no_ar", False)
    no_gather = debug.get("no_gather", False)
    stop_f0 = debug.get("stop_f0", False)
    shared_ag = debug.get("shared_ag", True)
    N, SLICE, PAD, NT = cfg["N"], cfg["SLICE"], cfg["PAD"], cfg["NT"]
    WT, SPANS = cfg["WT"], cfg["SPANS"]
    cap, blk_start = sched["cap"], sched["blk_start"]
    call_start, call_len = sched["call_start"], sched["call_len"]
    S, NCH, first_w = sched["S"], sched["NCH"], sched["first_w"]
    inv_nd = 1.0 / (float(N) * float(D))
    tailz = PAD - SLICE
    max_ln = max(call_len)
    nspans = len(SPANS)

    nc = bacc.Bacc("TRN2", target_bir_lowering=False, debug=False,
                   enable_asserts=False, num_devices=NCORES,
                   num_swdge_queues=4)

    t_xT = nc.dram_tensor("xT", [2, 128, PAD], BF16, kind="ExternalInput")
    t_idx = nc.dram_tensor("idx", [128, S // 16], I16, kind="ExternalInput")
    t_col = nc.dram_tensor("colv", [128, NCH], F32, kind="ExternalInput")
    t_en = nc.dram_tensor("env", [128, NCH], F32, kind="ExternalInput")
    t_l1w = nc.dram_tensor("lin1wT", [2, 128, 128], BF16, kind="ExternalInput")
    t_l1b = nc.dram_tensor("lin1b", [128, 1], F32, kind="ExternalInput")
    t_m1 = nc.dram_tensor("m1", [L, 128, 128], BF16, kind="ExternalInput")
    t_m2 = nc.dram_tensor("m2", [L, 128, 128], BF16, kind="ExternalInput")
    t_nw = nc.dram_tensor("nw", [128, 1], F32, kind="ExternalInput")
    t_nb = nc.dram_tensor("nb", [128, 1], F32, kind="ExternalInput")
    t_idb = nc.dram_tensor("identb", [128, 128], BF16, kind="ExternalInput")
    t_idf = nc.dram_tensor("identf", [128, 128], F32, kind="ExternalInput")
    t_iota = nc.dram_tensor("iota", [128, 128], F32, kind="ExternalInput")
    t_y = nc.dram_tensor("y", [PAD, 128], F32, kind="ExternalOutput")

    rg = [list(range(NCORES))]
    ag_space = "Shared" if shared_ag else "Local"

    with tile.TileContext(nc) as tc:
        with tc.tile_pool(name="res", bufs=1) as res, \
             tc.tile_pool(name="gp", bufs=2) as gp, \
             tc.tile_pool(name="bb", bufs=6) as bbp, \
             tc.tile_pool(name="hb", bufs=4) as hbp, \
             tc.tile_pool(name="scr", bufs=2) as scrp, \
             tc.tile_pool(name="xt", bufs=2) as xtp, \
             tc.tile_pool(name="fn", bufs=2) as fnp, \
             tc.tile_pool(name="tr", bufs=2) as trp, \
             tc.tile_pool(name="sv", bufs=2) as sv, \
             tc.tile_pool(name="psA", bufs=2, space="PSUM") as psA, \
             tc.tile_pool(name="psB", bufs=2, space="PSUM") as psB, \
             tc.tile_pool(name="psT", bufs=1, space="PSUM") as psT, \
             tc.tile_pool(name="psM", bufs=1, space="PSUM") as psM, \
             tc.tile_pool(name="dram", bufs=1, space="DRAM") as dram:

            f_slice = [[dram.tile([(b - a) * 128, 128], BF16,
                                  name=f"f_slice_{li}_{w}")
                        for w, (a, b) in enumerate(WT)]
                       for li in range(L)]
            f_full = [[dram.tile([NCORES * (b - a) * 128, 128], BF16,
                                 name=f"f_full_{li}_{w}",
                                 addr_space=ag_space)
                       for w, (a, b) in enumerate(WT)]
                      for li in range(L)]
            ar_in = dram.tile([1, 8], F32)
            ar_out = dram.tile([1, 8], F32)

            # ---- resident loads
            idx_sb = res.tile([128, S // 16], I16)
            nc.sync.dma_start(idx_sb[:], t_idx[:])
            col_sb = res.tile([128, NCH], F32)
            nc.sync.dma_start(col_sb[:], t_col[:])
            en_sb = res.tile([128, NCH], F32)
            nc.sync.dma_start(en_sb[:], t_en[:])
            iota_sb = res.tile([128, 128], F32)
            nc.sync.dma_start(iota_sb[:], t_iota[:])
            idb_sb = res.tile([128, 128], BF16)
            nc.sync.dma_start(idb_sb[:], t_idb[:])
            idf_sb = res.tile([128, 128], F32)
            nc.sync.dma_start(idf_sb[:], t_idf[:])
            l1w_sb = res.tile([128, 2, 128], BF16)
            nc.sync.dma_start(l1w_sb[:], t_l1w[:].rearrange("j k f -> k j f"))
            l1b_sb = res.tile([128, 1], F32)
            nc.sync.dma_start(l1b_sb[:], t_l1b[:])
            m1_sb = res.tile([128, L, 128], BF16)
            nc.sync.dma_start(m1_sb[:], t_m1[:].rearrange("l g f -> g l f"))
            m2_sb = res.tile([128, L, 128], BF16)
            nc.sync.dma_start(m2_sb[:], t_m2[:].rearrange("l g f -> g l f"))
            nw_sb = res.tile([128, 1], F32)
            nc.sync.dma_start(nw_sb[:], t_nw[:])
            nb_sb = res.tile([128, 1], F32)
            nc.sync.dma_start(nb_sb[:], t_nb[:])

            x0_sb = res.tile([128, NT, 128], BF16)
            h_sb = res.tile([128, NT, 128], F32)   # H accum, then holds P
            acc_s = res.tile([128, NT], F32)
            acc_q = res.tile([128, NT], F32)
            ones_c = res.tile([128, 1], F32)
            nc.vector.memset(ones_c[:], 1.0)
            ones_r = res.tile([1, 128], F32)
            nc.vector.memset(ones_r[:], 1.0)

            def store_window(li, w, last):
                """normalize+relu tiles of window w from h_sb (holding P),
                transpose to node-major, store to f_slice[li+1][w] (or t_y)."""
                a, b = WT[w]
                for s0 in range(a, b, 4):
                    s1 = min(s0 + 4, b)
                    gsz = s1 - s0
                    dt_ = F32 if last else BF16
                    idm = idf_sb if last else idb_sb
                    fn = fnp.tile([128, 4, 128], dt_, tag=f"fn{dt_}")
                    nc.scalar.activation(fn[:, :gsz, :],
                                         h_sb[:, s0:s1, :],
                                         AF.Relu, bias=bv[:], scale=scv[:])
                    tr_ps = psT.tile([128, 4, 128], dt_, tag="Tf" if last else "T")
                    for j in range(gsz):
                        nc.tensor.transpose(tr_ps[:, j, :], fn[:, j, :],
                                            idm[:])
                    trs = trp.tile([128, 4, 128], dt_, tag=f"tr{dt_}")
                    nc.vector.tensor_copy(trs[:, :gsz, :], tr_ps[:, :gsz, :])
                    dst = t_y if last else f_slice[li + 1][w]
                    o0 = s0 * 128 if last else (s0 - a) * 128
                    nc.sync.dma_start(
                        dst[o0:o0 + gsz * 128, :].rearrange(
                            "(j d) f -> d j f", j=gsz),
                        trs[:, :gsz, :])

            # ---------------- phase 0: f0 = relu(lin1(x)), store, sub-AGs
            for w, (a, b) in enumerate(WT):
                for s0 in range(a, b, 4):
                    s1 = min(s0 + 4, b)
                    gsz = s1 - s0
                    xt = xtp.tile([128, 2, 4 * 128], BF16, tag="xt")
                    nc.sync.dma_start(
                        xt[:, :, :gsz * 128],
                        t_xT[:, :, s0 * 128:s1 * 128].rearrange(
                            "j k d -> k j d"))
                    f0_ps = psB.tile([128, 4, 128], F32, tag="P")
                    nc.tensor.matmul(f0_ps[:, :gsz, :], l1w_sb[:, 0, :],
                                     xt[:, 0, :gsz * 128], start=True,
                                     stop=False)
                    nc.tensor.matmul(f0_ps[:, :gsz, :], l1w_sb[:, 1, :],
                                     xt[:, 1, :gsz * 128], start=False,
                                     stop=True)
                    nc.scalar.activation(x0_sb[:, s0:s1, :],
                                         f0_ps[:, :gsz, :],
                                         AF.Relu, bias=l1b_sb[:], scale=1.0)
                    if s1 == NT and tailz > 0:
                        nc.vector.memset(x0_sb[:, NT - 1, 128 - tailz:], 0.0)
                    tr_ps = psT.tile([128, 4, 128], BF16, tag="T")
                    for j in range(gsz):
                        nc.tensor.transpose(tr_ps[:, j, :],
                                            x0_sb[:, s0 + j, :], idb_sb[:])
                    trs = trp.tile([128, 4, 128], BF16, tag=f"tr{BF16}")
                    nc.vector.tensor_copy(trs[:, :gsz, :], tr_ps[:, :gsz, :])
                    nc.sync.dma_start(
                        f_slice[0][w][(s0 - a) * 128:(s1 - a) * 128,
                                      :].rearrange(
                            "(j d) f -> d j f", j=gsz),
                        trs[:, :gsz, :])
                nc.gpsimd.collective_compute(
                    "AllGather", AL.bypass, replica_groups=rg,
                    ins=[f_slice[0][w].opt()], outs=[f_full[0][w].opt()])
            if stop_f0:
                for w, (a, b) in enumerate(WT):
                    nc.gpsimd.dma_start(
                        t_y[a * 128:b * 128, :],
                        f_full[0][w][:(b - a) * 128, :])  # core0 slice, cast
                n_layers_eff = 0
            else:
                n_layers_eff = n_layers

            # ---------------- layers
            for li in range(n_layers_eff):
                last = li == L - 1
                for si, (w, s0, s1) in enumerate(SPANS):
                    a = call_start[si]
                    ln = call_len[si]
                    gt = gp.tile([128, max_ln // 128, 128], BF16, tag="g")
                    if ln:
                        if no_gather:
                            nc.vector.memset(gt[:, :ln // 128, :], 0.0)
                        else:
                            nc.gpsimd.dma_gather(
                                gt[:, :ln // 128, :], f_full[li][w][:, :],
                                idx_sb[:, a // 16:(a + ln) // 16],
                                ln, ln, 128, single_packet=False,
                                queue_num=si % 4)
                    for t in range(s0, s1):
                        nck = int(cap[t, w]) // 128
                        if nck == 0:
                            continue
                        cl0 = (blk_start[(t, w)] - a) // 128
                        cg0 = blk_start[(t, w)] // 128
                        h_ps = psA.tile([128, 128], F32, tag="H")
                        for k in range(nck):
                            bbt = bbp.tile([128, 128], BF16, tag="bb")
                            nc.vector.tensor_scalar(
                                bbt[:], iota_sb[:],
                                col_sb[:, cg0 + k:cg0 + k + 1],
                                en_sb[:, cg0 + k:cg0 + k + 1],
                                op0=AL.is_equal, op1=AL.mult)
                            nc.tensor.matmul(
                                h_ps[:], gt[:, cl0 + k, :], bbt[:],
                                start=(k == 0), stop=(k == nck - 1))
                        if w == first_w[t]:
                            nc.vector.tensor_copy(h_sb[:, t, :], h_ps[:])
                        else:
                            nc.vector.tensor_tensor(
                                h_sb[:, t, :], h_sb[:, t, :], h_ps[:],
                                op=AL.add)
                        if w == 3:
                            # H[t] complete: dense mixes + stats now
                            hbt = hbp.tile([128, 128], BF16, tag="hb")
                            nc.scalar.activation(hbt[:], h_sb[:, t, :],
                                                 AF.Copy)
                            p_ps = psB.tile([128, 4, 128], F32, tag="P")
                            nc.tensor.matmul(p_ps[:, 0, :], m1_sb[:, li, :],
                                             hbt[:], start=True, stop=False)
                            nc.tensor.matmul(p_ps[:, 0, :], m2_sb[:, li, :],
                                             x0_sb[:, t, :], start=False,
                                             stop=True)
                            nc.scalar.activation(
                                h_sb[:, t, :], p_ps[:, 0, :], AF.Copy,
                                accum_out=acc_s[:, t:t + 1])
                            scr = scrp.tile([128, 128], BF16, tag="scr")
                            nc.scalar.activation(scr[:], p_ps[:, 0, :],
                                                 AF.Square,
                                                 accum_out=acc_q[:, t:t + 1])

                # ---- global stats -> AllReduce -> scale/bias vectors
                tot = sv.tile([128, 2], F32, tag="tot")
                nc.vector.tensor_reduce(tot[:, 0:1], acc_s[:, :], axis=AX.X,
                                        op=AL.add)
                nc.vector.tensor_reduce(tot[:, 1:2], acc_q[:, :], axis=AX.X,
                                        op=AL.add)
                st_ps = psM.tile([128, 2], F32, tag="M")
                nc.tensor.matmul(st_ps[0:1, :], ones_c[:], tot[:],
                                 start=True, stop=True)
                st8 = sv.tile([1, 8], F32, tag="st8")
                nc.vector.memset(st8[:], 0.0)
                nc.vector.tensor_copy(st8[0:1, 0:2], st_ps[0:1, :])
                nc.sync.dma_start(ar_in[:], st8[:])
                if no_ar:
                    nc.sync.dma_start(ar_out[:], ar_in[:])
                else:
                    nc.gpsimd.collective_compute(
                        "AllReduce", AL.add, replica_groups=rg,
                        ins=[ar_in.opt()], outs=[ar_out.opt()])
                gs = sv.tile([1, 8], F32, tag="gs")
                nc.sync.dma_start(gs[:], ar_out[:])
                ms = sv.tile([1, 4], F32, tag="ms")
                nc.vector.tensor_scalar(ms[0:1, 0:1], gs[0:1, 0:1], inv_nd,
                                        None, op0=AL.mult)          # m
                nc.vector.tensor_scalar(ms[0:1, 1:2], gs[0:1, 1:2], inv_nd,
                                        None, op0=AL.mult)          # E[x^2]
                nc.vector.tensor_mul(ms[0:1, 2:3], ms[0:1, 0:1], ms[0:1, 0:1])
                nc.vector.tensor_sub(ms[0:1, 3:4], ms[0:1, 1:2], ms[0:1, 2:3])
                sq = sv.tile([1, 4], F32, tag="sq")
                nc.scalar.activation(sq[0:1, 0:1], ms[0:1, 3:4], AF.Sqrt)
                nc.vector.tensor_scalar(sq[0:1, 1:2], sq[0:1, 0:1], EPS, None,
                                        op0=AL.add)
                nc.vector.reciprocal(sq[0:1, 2:3], sq[0:1, 1:2])    # inv
                nc.vector.tensor_mul(sq[0:1, 3:4], sq[0:1, 2:3], ms[0:1, 0:1])
                pk = sv.tile([1, 2], F32, tag="pk")
                nc.vector.tensor_copy(pk[0:1, 0:1], sq[0:1, 2:3])
                nc.vector.tensor_copy(pk[0:1, 1:2], sq[0:1, 3:4])
                bc_ps = psM.tile([128, 2], F32, tag="M")
                nc.tensor.matmul(bc_ps[:], ones_r[:], pk[:],
                                 start=True, stop=True)
                bc = sv.tile([128, 2], F32, tag="bc")
                nc.vector.tensor_copy(bc[:], bc_ps[:])
                scv = sv.tile([128, 1], F32, tag="scv")
                nc.vector.tensor_mul(scv[:], bc[:, 0:1], nw_sb[:])
                bv1 = sv.tile([128, 1], F32, tag="bv1")
                nc.vector.tensor_mul(bv1[:], bc[:, 1:2], nw_sb[:])
                bv = sv.tile([128, 1], F32, tag="bv")
                nc.vector.tensor_sub(bv[:], nb_sb[:], bv1[:])

                # ---- normalize + relu + transpose + store + sub-AG
                for w in range(4):
                    store_window(li, w, last)
                    if not last:
                        nc.gpsimd.collective_compute(
                            "AllGather", AL.bypass, replica_groups=rg,
                            ins=[f_slice[li + 1][w].opt()],
                            outs=[f_full[li + 1][w].opt()])

    nc.compile()
    return nc


_last_results = None


def run(inputs, cfg, trace=False, debug=None):
    global _last_results
    sched, per_core, consts = preprocess(
        inputs["x"], inputs["edge_index"], inputs["lin1_w"], inputs["lin1_b"],
        inputs["w1"], inputs["w2"], inputs["norm_w"], inputs["norm_b"], cfg)
    nc = build(cfg, sched, debug=debug)
    in_maps = []
    for c in range(NCORES):
        m = dict(per_core[c])
        m.update(consts)
        in_maps.append(m)
    _last_results = run_bass_kernel_spmd(
        nc, in_maps, core_ids=list(range(NCORES)), trace=trace)
    SLICE = cfg["SLICE"]
    out = np.concatenate(
        [_last_results.results[c]["y"][:SLICE] for c in range(NCORES)], axis=0)
    return out.astype(np.float32)


def kernel(**inputs):
    return run(inputs, full_cfg(inputs["x"].shape[0]))


# revision 8
# speedup vs baseline: 1.0551x; 1.0551x over previous
"""GCNII backbone Bass/Trainium2 kernel — 8-core SPMD, v2.

Sharding: nodes row-partitioned across 8 cores (12500/core, padded to 12544
= 98 tiles of 128).  Edges live on the core that owns their *destination*
node.  Host-side index-only preprocessing builds a window-major,
destination-sorted, capacity-padded edge stream per core; the device does
everything float.

v2 structure (vs v1):
  * segment matrix B is built ON DEVICE per 128x128 chunk with one DVE
    dual-op tensor_scalar (is_equal by col, mult by enorm) from two small
    resident vectors — no 70MB/layer HBM stream of B.
  * f_full is laid out tile-major-interleaved and split into 4 window
    tensors f_full[w] (<=32768 rows each, int16-indexable).  Each window is
    AllGathered separately (Shared output) as soon as every core stored its
    slice piece, and the next layer's window-w gathers depend only on
    sub-AG w -> collectives pipeline behind compute.
  * window-major schedule: for w: gather spans of 14 dst tiles, chunk
    matmuls accumulate into PSUM per (tile,window), DVE folds into an SBUF
    f32 accumulator H; last window also feeds the dense P matmuls, stats,
    AllReduce, normalize, transpose, store, sub-AGs.
"""

import os
import sys

for _p in ("/opt/trn_rl_repo",):
    if _p not in sys.path:
        sys.path.insert(0, _p)

import math

import ml_dtypes
import numpy as np

import concourse.bacc as bacc
import concourse.bass as bass
import concourse.tile as tile
from concourse import mybir
from concourse.bass_utils import run_bass_kernel_spmd

F32 = mybir.dt.float32
BF16 = mybir.dt.bfloat16
I16 = mybir.dt.int16
AX = mybir.AxisListType
AL = mybir.AluOpType
AF = mybir.ActivationFunctionType

NCORES = 8
D = 128
DIN = 256
L = 4
ALPHA = 0.5
THETA = 1.0
EPS = 1e-5


def _mk_cfg(N, slice_, sp):
    pad = ((slice_ + 127) // 128) * 128
    nt = pad // 128
    # split tiles into 4 windows (gather source ranges; one sub-AG each)
    base = -(-nt // 4)
    wt = []
    t0 = 0
    for w in range(4):
        t1 = min(t0 + base, nt)
        wt.append((t0, t1))
        t0 = t1
    assert wt[-1][1] == nt
    for (a, b) in wt:
        assert (b - a) * 128 * NCORES <= 32768  # int16-indexable window
    # spans: dst tiles per gather call; every src window sweeps ALL dst tiles
    spans = []
    for w in range(4):
        s0 = 0
        while s0 < nt:
            s1 = min(s0 + sp, nt)
            spans.append((w, s0, s1))
            s0 = s1
    return dict(N=N, SLICE=slice_, PAD=pad, NT=nt, WT=wt, SPANS=spans)


def full_cfg(N=100000):
    return _mk_cfg(N, N // NCORES, 8)


def small_cfg():
    return _mk_cfg(8000, 1000, 2)


# ---------------------------------------------------------------- host prep
def preprocess(x, edge_index, lin1_w, lin1_b, w1, w2, norm_w, norm_b, cfg):
    N, SLICE, PAD, NT = cfg["N"], cfg["SLICE"], cfg["PAD"], cfg["NT"]
    WT, SPANS = cfg["WT"], cfg["SPANS"]

    src = np.asarray(edge_index[0], dtype=np.int64)
    dst = np.asarray(edge_index[1], dtype=np.int64)
    sl = np.arange(N, dtype=np.int64)
    srcA = np.concatenate([src, sl])
    dstA = np.concatenate([dst, sl])

    deg = np.bincount(dstA, minlength=N).astype(np.float64)
    dis = 1.0 / np.sqrt(deg)
    en = ((1.0 - ALPHA) * dis[srcA] * dis[dstA]).astype(np.float32)

    # source address in its window tensor: f_full[w][c*TW*128 + (t-t0)*128 + r]
    s_core = srcA // SLICE
    s_loc = srcA % SLICE
    s_tile = s_loc // 128
    s_row = s_loc % 128
    t0s = np.zeros(NT, np.int64)
    wofs = np.zeros(NT, np.int64)  # window id per tile
    for w, (a, b) in enumerate(WT):
        wofs[a:b] = w
        t0s[a:b] = a
    twsize = np.array([b - a for (a, b) in WT], np.int64)
    s_w = wofs[s_tile]
    addr = (s_core * twsize[s_w] + (s_tile - t0s[s_tile])) * 128 + s_row
    assert addr.max() < 32768

    core = dstA // SLICE
    lt = (dstA % SLICE) // 128
    colr = ((dstA % SLICE) % 128).astype(np.float32)

    # per-(core, dst tile, window) counts -> shared static caps (mult of 128)
    blk = (core * NT + lt) * 4 + s_w
    cnt = np.bincount(blk, minlength=NCORES * NT * 4).reshape(NCORES, NT, 4)
    cap = (np.ceil(cnt.max(axis=0) / 128).astype(np.int64)) * 128  # [NT, 4]

    # stream block order: src-window-major, then dst tile
    border = [(t, w) for w in range(4) for t in range(NT)]
    blk_of = {tw: i for i, tw in enumerate(border)}
    blk_len = np.array([cap[t, w] for (t, w) in border], dtype=np.int64)
    blk_start_arr = np.concatenate([[0], np.cumsum(blk_len)])
    S_total = int(blk_start_arr[-1])
    NCH = S_total // 128
    blk_start = {tw: int(blk_start_arr[i]) for i, tw in enumerate(border)}

    # gather calls: one per span (w, s0, s1)
    call_start = [blk_start[(s0, w)] for (w, s0, s1) in SPANS]
    call_len = [int(cap[s0:s1, w].sum()) for (w, s0, s1) in SPANS]
    first_w = {}  # first window with edges, per tile
    for t in range(NT):
        first_w[t] = int(np.nonzero(cap[t, :])[0][0])

    sched = dict(cap=cap, blk_start=blk_start, call_start=call_start,
                 call_len=call_len, S=S_total, NCH=NCH, first_w=first_w)

    # per-core streams
    per_core = []
    bidx_all = np.array([blk_of[(int(t), int(w))] for t, w in zip(lt, s_w)],
                        dtype=np.int64)
    for c in range(NCORES):
        m = core == c
        bi = bidx_all[m]
        order = np.argsort(bi, kind="stable")
        bi_s = bi[order]
        cnts = np.bincount(bi_s, minlength=len(border))
        starts_sorted = np.concatenate([[0], np.cumsum(cnts)])[:-1]
        rank = np.arange(len(bi_s)) - starts_sorted[bi_s]
        pos = blk_start_arr[bi_s] + rank

        idx_s = np.zeros(S_total, np.int64)
        col_s = np.zeros(S_total, np.int64)
        en_s = np.zeros(S_total, np.float32)
        idx_s[pos] = addr[m][order]
        col_s[pos] = colr[m][order].astype(np.int64)
        en_s[pos] = en[m][order]

        # idx packed per gather call: wrap 16 partitions, replicate x8
        idxp = np.zeros((16, S_total // 16), np.int16)
        for a, ln in zip(call_start, call_len):
            if ln == 0:
                continue
            seg = idx_s[a:a + ln].astype(np.int16)
            idxp[:, a // 16:(a + ln) // 16] = seg.reshape(ln // 16, 16).T
        idxp = np.tile(idxp, (NCORES, 1))

        # chunk-major col / enorm vectors for on-device B build
        colp = col_s.reshape(NCH, 128).T.astype(np.float32)
        enp = en_s.reshape(NCH, 128).T.astype(np.float32)
        colp = np.ascontiguousarray(colp)
        enp = np.ascontiguousarray(enp)

        # x slice, transposed+packed on host: xT[j,k,d] = x[row d, 128j+k]
        xs = np.zeros((PAD, DIN), np.float32)
        xs[:SLICE] = np.asarray(x[c * SLICE:(c + 1) * SLICE], np.float32)
        xT = np.ascontiguousarray(
            xs.T.reshape(2, 128, PAD)).astype(ml_dtypes.bfloat16)

        per_core.append(dict(idx=idxp, colv=colp, env=enp, xT=xT))

    # weights
    lw = np.asarray(lin1_w, np.float32)          # [128, 256]
    lin1wT = np.ascontiguousarray(lw.T.reshape(2, 128, 128)).astype(
        ml_dtypes.bfloat16)
    m1 = np.zeros((L, 128, 128), np.float32)
    m2 = np.zeros((L, 128, 128), np.float32)
    eye = np.eye(128, dtype=np.float32)
    for li in range(L):
        beta = float(np.log(THETA / (li + 1) + 1.0))
        m1[li] = (1.0 - beta) * eye + beta * np.asarray(w1[li], np.float32)
        m2[li] = ALPHA * ((1.0 - beta) * eye + beta * np.asarray(w2[li], np.float32))
    iota = np.tile(np.arange(128, dtype=np.float32), (128, 1))
    consts = dict(
        lin1wT=lin1wT,
        lin1b=np.asarray(lin1_b, np.float32).reshape(128, 1),
        m1=m1.astype(ml_dtypes.bfloat16), m2=m2.astype(ml_dtypes.bfloat16),
        nw=np.asarray(norm_w, np.float32).reshape(128, 1),
        nb=np.asarray(norm_b, np.float32).reshape(128, 1),
        identb=np.eye(128, dtype=np.float32).astype(ml_dtypes.bfloat16),
        identf=np.eye(128, dtype=np.float32),
        iota=iota,
    )
    return sched, per_core, consts


# ---------------------------------------------------------------- device IR
def build(cfg, sched, debug=None):
    debug = debug or {}
    n_layers = debug.get("n_layers", L)
    no_ar = debug.get("no_ar", False)
    no_gather = debug.get("no_gather", False)
    stop_f0 = debug.get("stop_f0", False)
    shared_ag = debug.get("shared_ag", False)
    N, SLICE, PAD, NT = cfg["N"], cfg["SLICE"], cfg["PAD"], cfg["NT"]
    WT, SPANS = cfg["WT"], cfg["SPANS"]
    cap, blk_start = sched["cap"], sched["blk_start"]
    call_start, call_len = sched["call_start"], sched["call_len"]
    S, NCH, first_w = sched["S"], sched["NCH"], sched["first_w"]
    inv_nd = 1.0 / (float(N) * float(D))
    tailz = PAD - SLICE
    max_ln = max(call_len)
    nspans = len(SPANS)

    nc = bacc.Bacc("TRN2", target_bir_lowering=False, debug=False,
                   enable_asserts=False, num_devices=NCORES,
                   num_swdge_queues=4)

    t_xT = nc.dram_tensor("xT", [2, 128, PAD], BF16, kind="ExternalInput")
    t_idx = nc.dram_tensor("idx", [128, S // 16], I16, kind="ExternalInput")
    t_col = nc.dram_tensor("colv", [128, NCH], F32, kind="ExternalInput")
    t_en = nc.dram_tensor("env", [128, NCH], F32, kind="ExternalInput")
    t_l1w = nc.dram_tensor("lin1wT", [2, 128, 128], BF16, kind="ExternalInput")
    t_l1b = nc.dram_tensor("lin1b", [128, 1], F32, kind="ExternalInput")
    t_m1 = nc.dram_tensor("m1", [L, 128, 128], BF16, kind="ExternalInput")
    t_m2 = nc.dram_tensor("m2", [L, 128, 128], BF16, kind="ExternalInput")
    t_nw = nc.dram_tensor("nw", [128, 1], F32, kind="ExternalInput")
    t_nb = nc.dram_tensor("nb", [128, 1], F32, kind="ExternalInput")
    t_idb = nc.dram_tensor("identb", [128, 128], BF16, kind="ExternalInput")
    t_idf = nc.dram_tensor("identf", [128, 128], F32, kind="ExternalInput")
    t_iota = nc.dram_tensor("iota", [128, 128], F32, kind="ExternalInput")
    t_y = nc.dram_tensor("y", [PAD, 128], F32, kind="ExternalOutput")

    rg = [list(range(NCORES))]
    ag_space = "Shared" if shared_ag else "Local"

    with tile.TileContext(nc) as tc:
        with tc.tile_pool(name="res", bufs=1) as res, \
             tc.tile_pool(name="gp", bufs=3) as gp, \
             tc.tile_pool(name="bb", bufs=12) as bbp, \
             tc.tile_pool(name="hb", bufs=4) as hbp, \
             tc.tile_pool(name="scr", bufs=2) as scrp, \
             tc.tile_pool(name="xt", bufs=2) as xtp, \
             tc.tile_pool(name="fn", bufs=2) as fnp, \
             tc.tile_pool(name="tr", bufs=2) as trp, \
             tc.tile_pool(name="sv", bufs=2) as sv, \
             tc.tile_pool(name="psA", bufs=1, space="PSUM") as psA, \
             tc.tile_pool(name="psB", bufs=2, space="PSUM") as psB, \
             tc.tile_pool(name="psT", bufs=1, space="PSUM") as psT, \
             tc.tile_pool(name="psM", bufs=1, space="PSUM") as psM, \
             tc.tile_pool(name="dram", bufs=1, space="DRAM") as dram:

            f_slice = [[dram.tile([(b - a) * 128, 128], BF16,
                                  name=f"f_slice_{li}_{w}")
                        for w, (a, b) in enumerate(WT)]
                       for li in range(L)]
            f_full = [[dram.tile([NCORES * (b - a) * 128, 128], BF16,
                                 name=f"f_full_{li}_{w}",
                                 addr_space=ag_space)
                       for w, (a, b) in enumerate(WT)]
                      for li in range(L)]
            ar_in = dram.tile([1, 8], F32)
            ar_out = dram.tile([1, 8], F32)

            # ---- resident loads
            idx_sb = res.tile([128, S // 16], I16)
            nc.sync.dma_start(idx_sb[:], t_idx[:])
            col_sb = res.tile([128, NCH], F32)
            nc.sync.dma_start(col_sb[:], t_col[:])
            en_sb = res.tile([128, NCH], F32)
            nc.sync.dma_start(en_sb[:], t_en[:])
            iota_sb = res.tile([128, 128], F32)
            nc.sync.dma_start(iota_sb[:], t_iota[:])
            idb_sb = res.tile([128, 128], BF16)
            nc.sync.dma_start(idb_sb[:], t_idb[:])
            idf_sb = res.tile([128, 128], F32)
            nc.sync.dma_start(idf_sb[:], t_idf[:])
            l1w_sb = res.tile([128, 2, 128], BF16)
            nc.sync.dma_start(l1w_sb[:], t_l1w[:].rearrange("j k f -> k j f"))
            l1b_sb = res.tile([128, 1], F32)
            nc.sync.dma_start(l1b_sb[:], t_l1b[:])
            m1_sb = res.tile([128, L, 128], BF16)
            nc.sync.dma_start(m1_sb[:], t_m1[:].rearrange("l g f -> g l f"))
            m2_sb = res.tile([128, L, 128], BF16)
            nc.sync.dma_start(m2_sb[:], t_m2[:].rearrange("l g f -> g l f"))
            nw_sb = res.tile([128, 1], F32)
            nc.sync.dma_start(nw_sb[:], t_nw[:])
            nb_sb = res.tile([128, 1], F32)
            nc.sync.dma_start(nb_sb[:], t_nb[:])

            h_ps_all = psA.tile([128, 8, 128], F32, tag="H")
            x0_sb = res.tile([128, NT, 128], BF16)
            h_sb = res.tile([128, NT, 128], F32)   # H accum, then holds P
            acc_s = res.tile([128, NT], F32)
            acc_q = res.tile([128, NT], F32)
            ones_c = res.tile([128, 1], F32)
            nc.vector.memset(ones_c[:], 1.0)
            ones_r = res.tile([1, 128], F32)
            nc.vector.memset(ones_r[:], 1.0)

            def store_window(li, w, last):
                """normalize+relu tiles of window w from h_sb (holding P),
                transpose to node-major, store to f_slice[li+1][w] (or t_y)."""
                a, b = WT[w]
                for s0 in range(a, b, 4):
                    s1 = min(s0 + 4, b)
                    gsz = s1 - s0
                    dt_ = F32 if last else BF16
                    idm = idf_sb if last else idb_sb
                    fn = fnp.tile([128, 4, 128], dt_, tag=f"fn{dt_}")
                    nc.scalar.activation(fn[:, :gsz, :],
                                         h_sb[:, s0:s1, :],
                                         AF.Relu, bias=bv[:], scale=scv[:])
                    tr_ps = psT.tile([128, 4, 128], dt_, tag="Tf" if last else "T")
                    for j in range(gsz):
                        nc.tensor.transpose(tr_ps[:, j, :], fn[:, j, :],
                                            idm[:])
                    trs = trp.tile([128, 4, 128], dt_, tag=f"tr{dt_}")
                    nc.vector.tensor_copy(trs[:, :gsz, :], tr_ps[:, :gsz, :])
                    dst = t_y if last else f_slice[li + 1][w]
                    o0 = s0 * 128 if last else (s0 - a) * 128
                    nc.sync.dma_start(
                        dst[o0:o0 + gsz * 128, :].rearrange(
                            "(j d) f -> d j f", j=gsz),
                        trs[:, :gsz, :])

            # ---------------- phase 0: f0 = relu(lin1(x)), store, sub-AGs
            for w, (a, b) in enumerate(WT):
                for s0 in range(a, b, 4):
                    s1 = min(s0 + 4, b)
                    gsz = s1 - s0
                    xt = xtp.tile([128, 2, 4 * 128], BF16, tag="xt")
                    nc.sync.dma_start(
                        xt[:, :, :gsz * 128],
                        t_xT[:, :, s0 * 128:s1 * 128].rearrange(
                            "j k d -> k j d"))
                    f0_ps = psB.tile([128, 4, 128], F32, tag="P")
                    nc.tensor.matmul(f0_ps[:, :gsz, :], l1w_sb[:, 0, :],
                                     xt[:, 0, :gsz * 128], start=True,
                                     stop=False)
                    nc.tensor.matmul(f0_ps[:, :gsz, :], l1w_sb[:, 1, :],
                                     xt[:, 1, :gsz * 128], start=False,
                                     stop=True)
                    nc.scalar.activation(x0_sb[:, s0:s1, :],
                                         f0_ps[:, :gsz, :],
                                         AF.Relu, bias=l1b_sb[:], scale=1.0)
                    if s1 == NT and tailz > 0:
                        nc.vector.memset(x0_sb[:, NT - 1, 128 - tailz:], 0.0)
                    tr_ps = psT.tile([128, 4, 128], BF16, tag="T")
                    for j in range(gsz):
                        nc.tensor.transpose(tr_ps[:, j, :],
                                            x0_sb[:, s0 + j, :], idb_sb[:])
                    trs = trp.tile([128, 4, 128], BF16, tag=f"tr{BF16}")
                    nc.vector.tensor_copy(trs[:, :gsz, :], tr_ps[:, :gsz, :])
                    nc.sync.dma_start(
                        f_slice[0][w][(s0 - a) * 128:(s1 - a) * 128,
                                      :].rearrange(
                            "(j d) f -> d j f", j=gsz),
                        trs[:, :gsz, :])
                nc.gpsimd.collective_compute(
                    "AllGather", AL.bypass, replica_groups=rg,
                    ins=[f_slice[0][w].opt()], outs=[f_full[0][w].opt()])
            if stop_f0:
                for w, (a, b) in enumerate(WT):
                    nc.gpsimd.dma_start(
                        t_y[a * 128:b * 128, :],
                        f_full[0][w][:(b - a) * 128, :])  # core0 slice, cast
                n_layers_eff = 0
            else:
                n_layers_eff = n_layers

            # ---------------- layers
            hrot = [0]
            for li in range(n_layers_eff):
                last = li == L - 1
                for si, (w, s0, s1) in enumerate(SPANS):
                    a = call_start[si]
                    ln = call_len[si]
                    gt = gp.tile([128, max_ln // 128, 128], BF16, tag="g")
                    if ln:
                        if no_gather:
                            nc.vector.memset(gt[:, :ln // 128, :], 0.0)
                        else:
                            nc.gpsimd.dma_gather(
                                gt[:, :ln // 128, :], f_full[li][w][:, :],
                                idx_sb[:, a // 16:(a + ln) // 16],
                                ln, ln, 128, single_packet=False,
                                queue_num=si % 4)
                    for t in range(s0, s1):
                        nck = int(cap[t, w]) // 128
                        if nck == 0:
                            continue
                        cl0 = (blk_start[(t, w)] - a) // 128
                        cg0 = blk_start[(t, w)] // 128
                        h_ps = h_ps_all[:, hrot[0] % 8, :]
                        hrot[0] += 1
                        for k in range(nck):
                            bbt = bbp.tile([128, 128], BF16, tag="bb")
                            nc.vector.tensor_scalar(
                                bbt[:], iota_sb[:],
                                col_sb[:, cg0 + k:cg0 + k + 1],
                                en_sb[:, cg0 + k:cg0 + k + 1],
                                op0=AL.is_equal, op1=AL.mult)
                            nc.tensor.matmul(
                                h_ps[:], gt[:, cl0 + k, :], bbt[:],
                                start=(k == 0), stop=(k == nck - 1))
                        if w == first_w[t]:
                            nc.vector.tensor_copy(h_sb[:, t, :], h_ps[:])
                        else:
                            nc.vector.tensor_tensor(
                                h_sb[:, t, :], h_sb[:, t, :], h_ps[:],
                                op=AL.add)
                        if w == 3:
                            # H[t] complete: dense mixes + stats now
                            hbt = hbp.tile([128, 128], BF16, tag="hb")
                            nc.scalar.activation(hbt[:], h_sb[:, t, :],
                                                 AF.Copy)
                            p_ps = psB.tile([128, 4, 128], F32, tag="P")
                            nc.tensor.matmul(p_ps[:, 0, :], m1_sb[:, li, :],
                                             hbt[:], start=True, stop=False)
                            nc.tensor.matmul(p_ps[:, 0, :], m2_sb[:, li, :],
                                             x0_sb[:, t, :], start=False,
                                             stop=True)
                            nc.scalar.activation(
                                h_sb[:, t, :], p_ps[:, 0, :], AF.Copy,
                                accum_out=acc_s[:, t:t + 1])
                            scr = scrp.tile([128, 128], BF16, tag="scr")
                            nc.scalar.activation(scr[:], p_ps[:, 0, :],
                                                 AF.Square,
                                                 accum_out=acc_q[:, t:t + 1])

                # ---- global stats -> AllReduce -> scale/bias vectors
                tot = sv.tile([128, 2], F32, tag="tot")
                nc.vector.tensor_reduce(tot[:, 0:1], acc_s[:, :], axis=AX.X,
                                        op=AL.add)
                nc.vector.tensor_reduce(tot[:, 1:2], acc_q[:, :], axis=AX.X,
                                        op=AL.add)
                st_ps = psM.tile([128, 2], F32, tag="M")
                nc.tensor.matmul(st_ps[0:1, :], ones_c[:], tot[:],
                                 start=True, stop=True)
                st8 = sv.tile([1, 8], F32, tag="st8")
                nc.vector.memset(st8[:], 0.0)
                nc.vector.tensor_copy(st8[0:1, 0:2], st_ps[0:1, :])
                nc.sync.dma_start(ar_in[:], st8[:])
                if no_ar:
                    nc.sync.dma_start(ar_out[:], ar_in[:])
                else:
                    nc.gpsimd.collective_compute(
                        "AllReduce", AL.add, replica_groups=rg,
                        ins=[ar_in.opt()], outs=[ar_out.opt()])
                gs = sv.tile([1, 8], F32, tag="gs")
                nc.sync.dma_start(gs[:], ar_out[:])
                ms = sv.tile([1, 4], F32, tag="ms")
                nc.vector.tensor_scalar(ms[0:1, 0:1], gs[0:1, 0:1], inv_nd,
                                        None, op0=AL.mult)          # m
                nc.vector.tensor_scalar(ms[0:1, 1:2], gs[0:1, 1:2], inv_nd,
                                        None, op0=AL.mult)          # E[x^2]
                nc.vector.tensor_mul(ms[0:1, 2:3], ms[0:1, 0:1], ms[0:1, 0:1])
                nc.vector.tensor_sub(ms[0:1, 3:4], ms[0:1, 1:2], ms[0:1, 2:3])
                sq = sv.tile([1, 4], F32, tag="sq")
                nc.scalar.activation(sq[0:1, 0:1], ms[0:1, 3:4], AF.Sqrt)
                nc.vector.tensor_scalar(sq[0:1, 1:2], sq[0:1, 0:1], EPS, None,
                                        op0=AL.add)
                nc.vector.reciprocal(sq[0:1, 2:3], sq[0:1, 1:2])    # inv
                nc.vector.tensor_mul(sq[0:1, 3:4], sq[0:1, 2:3], ms[0:1, 0:1])
                pk = sv.tile([1, 2], F32, tag="pk")
                nc.vector.tensor_copy(pk[0:1, 0:1], sq[0:1, 2:3])
                nc.vector.tensor_copy(pk[0:1, 1:2], sq[0:1, 3:4])
                bc_ps = psM.tile([128, 2], F32, tag="M")
                nc.tensor.matmul(bc_ps[:], ones_r[:], pk[:],
                                 start=True, stop=True)
                bc = sv.tile([128, 2], F32, tag="bc")
                nc.vector.tensor_copy(bc[:], bc_ps[:])
                scv = sv.tile([128, 1], F32, tag="scv")
                nc.vector.tensor_mul(scv[:], bc[:, 0:1], nw_sb[:])
                bv1 = sv.tile([128, 1], F32, tag="bv1")
                nc.vector.tensor_mul(bv1[:], bc[:, 1:2], nw_sb[:])
                bv = sv.tile([128, 1], F32, tag="bv")
                nc.vector.tensor_sub(bv[:], nb_sb[:], bv1[:])

                # ---- normalize + relu + transpose + store + sub-AG
                for w in range(4):
                    store_window(li, w, last)
                    if not last:
                        nc.gpsimd.collective_compute(
                            "AllGather", AL.bypass, replica_groups=rg,
                            ins=[f_slice[li + 1][w].opt()],
                            outs=[f_full[li + 1][w].opt()])

    nc.compile()
    return nc


_last_results = None


def run(inputs, cfg, trace=False, debug=None):
    global _last_results
    sched, per_core, consts = preprocess(
        inputs["x"], inputs["edge_index"], inputs["lin1_w"], inputs["lin1_b"],
        inputs["w1"], inputs["w2"], inputs["norm_w"], inputs["norm_b"], cfg)
    nc = build(cfg, sched, debug=debug)
    in_maps = []
    for c in range(NCORES):
        m = dict(per_core[c])
        m.update(consts)
        in_maps.append(m)
    _last_results = run_bass_kernel_spmd(
        nc, in_maps, core_ids=list(range(NCORES)), trace=trace)
    SLICE = cfg["SLICE"]
    out = np.concatenate(
        [_last_results.results[c]["y"][:SLICE] for c in range(NCORES)], axis=0)
    return out.astype(np.float32)


def kernel(**inputs):
    return run(inputs, full_cfg(inputs["x"].shape[0]))


# revision 12
# speedup vs baseline: 1.3017x; 1.2337x over previous
"""GCNII backbone Bass/Trainium2 kernel — 8-core SPMD, v2.

Sharding: nodes row-partitioned across 8 cores (12500/core, padded to 12544
= 98 tiles of 128).  Edges live on the core that owns their *destination*
node.  Host-side index-only preprocessing builds a window-major,
destination-sorted, capacity-padded edge stream per core; the device does
everything float.

v2 structure (vs v1):
  * segment matrix B is built ON DEVICE per 128x128 chunk with one DVE
    dual-op tensor_scalar (is_equal by col, mult by enorm) from two small
    resident vectors — no 70MB/layer HBM stream of B.
  * f_full is laid out tile-major-interleaved and split into 4 window
    tensors f_full[w] (<=32768 rows each, int16-indexable).  Each window is
    AllGathered separately (Shared output) as soon as every core stored its
    slice piece, and the next layer's window-w gathers depend only on
    sub-AG w -> collectives pipeline behind compute.
  * window-major schedule: for w: gather spans of 14 dst tiles, chunk
    matmuls accumulate into PSUM per (tile,window), DVE folds into an SBUF
    f32 accumulator H; last window also feeds the dense P matmuls, stats,
    AllReduce, normalize, transpose, store, sub-AGs.
"""

import os
import sys

for _p in ("/opt/trn_rl_repo",):
    if _p not in sys.path:
        sys.path.insert(0, _p)

import math

import ml_dtypes
import numpy as np

import concourse.bacc as bacc
import concourse.bass as bass
import concourse.tile as tile
from concourse import mybir
from concourse.bass_utils import run_bass_kernel_spmd


def _register_en_onehot():
    """Custom fused DVE op: out[p,k] = (in0[p,k] == s0[p]) * s1[p].

    The stock dual-op tensor_scalar with two pointer scalars measures
    ~1.8us per [128,128] chunk on HW (scalars re-fetched per column); a
    fused custom-DVE op loads C0/C1 once and streams at full rate.
    """
    from concourse import dve_ops
    from concourse.dve_spec import Spec, Src0, C0, C1, eq, lower
    from concourse.dve_ops import has_src1
    from concourse.dve_uop import DveOpSpec
    from concourse.dve_table_gen import dve_ver_for

    name = "EN_ONEHOT"
    for op in dve_ops.OPS:
        if op.name == name:
            return op
    spec = Spec(
        body=eq(Src0, C0) * C1,
        reference=lambda in0, in1, s0, s1, imm2: ((in0 == s0) * s1),
    )
    row = dve_ops._CUSTOM_DVE_ROW_BASE + len(dve_ops.OPS)
    assert row < 0x20
    dve_ops._SUB_OPCODE_FOR_NAME[name] = row
    shas = {}
    for ver in ("v3", "v4"):
        try:
            dos = DveOpSpec(name=name, opcode=row, uops=lower(spec, ver=ver),
                            rd1_en=has_src1(spec))
            shas[ver] = dos.sha(ver)
        except Exception:
            pass
    op = dve_ops.DveOp(name, spec, subdim=False, uops_sha=shas)
    dve_ops.OPS.append(op)
    dve_ops.CUSTOM_DVE_SPECS[name] = spec
    return op


EN_ONEHOT = _register_en_onehot()

F32 = mybir.dt.float32
BF16 = mybir.dt.bfloat16
I16 = mybir.dt.int16
AX = mybir.AxisListType
AL = mybir.AluOpType
AF = mybir.ActivationFunctionType

NCORES = 8
D = 128
DIN = 256
L = 4
ALPHA = 0.5
THETA = 1.0
EPS = 1e-5


def _mk_cfg(N, slice_, sp):
    pad = ((slice_ + 127) // 128) * 128
    nt = pad // 128
    # split tiles into 4 windows (gather source ranges; one sub-AG each)
    base = -(-nt // 4)
    wt = []
    t0 = 0
    for w in range(4):
        t1 = min(t0 + base, nt)
        wt.append((t0, t1))
        t0 = t1
    assert wt[-1][1] == nt
    for (a, b) in wt:
        assert (b - a) * 128 * NCORES <= 32768  # int16-indexable window
    # spans: dst tiles per gather call; every src window sweeps ALL dst tiles
    spans = []
    for w in range(4):
        s0 = 0
        while s0 < nt:
            s1 = min(s0 + sp, nt)
            spans.append((w, s0, s1))
            s0 = s1
    return dict(N=N, SLICE=slice_, PAD=pad, NT=nt, WT=wt, SPANS=spans)


def full_cfg(N=100000):
    return _mk_cfg(N, N // NCORES, 8)


def small_cfg():
    return _mk_cfg(8000, 1000, 2)


# ---------------------------------------------------------------- host prep
def preprocess(x, edge_index, lin1_w, lin1_b, w1, w2, norm_w, norm_b, cfg):
    N, SLICE, PAD, NT = cfg["N"], cfg["SLICE"], cfg["PAD"], cfg["NT"]
    WT, SPANS = cfg["WT"], cfg["SPANS"]

    src = np.asarray(edge_index[0], dtype=np.int64)
    dst = np.asarray(edge_index[1], dtype=np.int64)
    sl = np.arange(N, dtype=np.int64)
    srcA = np.concatenate([src, sl])
    dstA = np.concatenate([dst, sl])

    deg = np.bincount(dstA, minlength=N).astype(np.float64)
    dis = 1.0 / np.sqrt(deg)
    en = ((1.0 - ALPHA) * dis[srcA] * dis[dstA]).astype(np.float32)

    # source address in its window tensor: f_full[w][c*TW*128 + (t-t0)*128 + r]
    s_core = srcA // SLICE
    s_loc = srcA % SLICE
    s_tile = s_loc // 128
    s_row = s_loc % 128
    t0s = np.zeros(NT, np.int64)
    wofs = np.zeros(NT, np.int64)  # window id per tile
    for w, (a, b) in enumerate(WT):
        wofs[a:b] = w
        t0s[a:b] = a
    twsize = np.array([b - a for (a, b) in WT], np.int64)
    s_w = wofs[s_tile]
    addr = (s_core * twsize[s_w] + (s_tile - t0s[s_tile])) * 128 + s_row
    assert addr.max() < 32768

    core = dstA // SLICE
    lt = (dstA % SLICE) // 128
    colr = ((dstA % SLICE) % 128).astype(np.float32)

    # per-(core, dst tile, window) counts -> shared static caps (mult of 128)
    blk = (core * NT + lt) * 4 + s_w
    cnt = np.bincount(blk, minlength=NCORES * NT * 4).reshape(NCORES, NT, 4)
    cap = (np.ceil(cnt.max(axis=0) / 128).astype(np.int64)) * 128  # [NT, 4]

    # stream block order: src-window-major, then dst tile
    border = [(t, w) for w in range(4) for t in range(NT)]
    blk_of = {tw: i for i, tw in enumerate(border)}
    blk_len = np.array([cap[t, w] for (t, w) in border], dtype=np.int64)
    blk_start_arr = np.concatenate([[0], np.cumsum(blk_len)])
    S_total = int(blk_start_arr[-1])
    NCH = S_total // 128
    blk_start = {tw: int(blk_start_arr[i]) for i, tw in enumerate(border)}

    # gather calls: one per span (w, s0, s1)
    call_start = [blk_start[(s0, w)] for (w, s0, s1) in SPANS]
    call_len = [int(cap[s0:s1, w].sum()) for (w, s0, s1) in SPANS]
    first_w = {}  # first window with edges, per tile
    for t in range(NT):
        first_w[t] = int(np.nonzero(cap[t, :])[0][0])

    sched = dict(cap=cap, blk_start=blk_start, call_start=call_start,
                 call_len=call_len, S=S_total, NCH=NCH, first_w=first_w)

    # per-core streams
    per_core = []
    bidx_all = np.array([blk_of[(int(t), int(w))] for t, w in zip(lt, s_w)],
                        dtype=np.int64)
    for c in range(NCORES):
        m = core == c
        bi = bidx_all[m]
        order = np.argsort(bi, kind="stable")
        bi_s = bi[order]
        cnts = np.bincount(bi_s, minlength=len(border))
        starts_sorted = np.concatenate([[0], np.cumsum(cnts)])[:-1]
        rank = np.arange(len(bi_s)) - starts_sorted[bi_s]
        pos = blk_start_arr[bi_s] + rank

        idx_s = np.zeros(S_total, np.int64)
        col_s = np.zeros(S_total, np.int64)
        en_s = np.zeros(S_total, np.float32)
        idx_s[pos] = addr[m][order]
        col_s[pos] = colr[m][order].astype(np.int64)
        en_s[pos] = en[m][order]

        # idx packed per gather call: wrap 16 partitions, replicate x8
        idxp = np.zeros((16, S_total // 16), np.int16)
        for a, ln in zip(call_start, call_len):
            if ln == 0:
                continue
            seg = idx_s[a:a + ln].astype(np.int16)
            idxp[:, a // 16:(a + ln) // 16] = seg.reshape(ln // 16, 16).T
        idxp = np.tile(idxp, (NCORES, 1))

        # host-built segment matrix B, streamed from HBM on device:
        # B[p, c, d] = enorm of edge (c*128+p) if its col == d else 0
        Bm = np.zeros((NCH, 128, 128), np.float32)
        Bm[np.arange(S_total) // 128, np.arange(S_total) % 128,
           col_s] = en_s
        Bm = np.ascontiguousarray(Bm.transpose(1, 0, 2)).astype(
            ml_dtypes.bfloat16)

        # x slice, transposed+packed on host: xT[j,k,d] = x[row d, 128j+k]
        xs = np.zeros((PAD, DIN), np.float32)
        xs[:SLICE] = np.asarray(x[c * SLICE:(c + 1) * SLICE], np.float32)
        xT = np.ascontiguousarray(
            xs.T.reshape(2, 128, PAD)).astype(ml_dtypes.bfloat16)

        per_core.append(dict(idx=idxp, bmat=Bm, xT=xT))

    # weights
    lw = np.asarray(lin1_w, np.float32)          # [128, 256]
    lin1wT = np.ascontiguousarray(lw.T.reshape(2, 128, 128)).astype(
        ml_dtypes.bfloat16)
    m1 = np.zeros((L, 128, 128), np.float32)
    m2 = np.zeros((L, 128, 128), np.float32)
    eye = np.eye(128, dtype=np.float32)
    for li in range(L):
        beta = float(np.log(THETA / (li + 1) + 1.0))
        m1[li] = (1.0 - beta) * eye + beta * np.asarray(w1[li], np.float32)
        m2[li] = ALPHA * ((1.0 - beta) * eye + beta * np.asarray(w2[li], np.float32))
    consts = dict(
        lin1wT=lin1wT,
        lin1b=np.asarray(lin1_b, np.float32).reshape(128, 1),
        m1=m1.astype(ml_dtypes.bfloat16), m2=m2.astype(ml_dtypes.bfloat16),
        nw=np.asarray(norm_w, np.float32).reshape(128, 1),
        nb=np.asarray(norm_b, np.float32).reshape(128, 1),
        identb=np.eye(128, dtype=np.float32).astype(ml_dtypes.bfloat16),
        identf=np.eye(128, dtype=np.float32),
    )
    return sched, per_core, consts


# ---------------------------------------------------------------- device IR
def build(cfg, sched, debug=None):
    debug = debug or {}
    n_layers = debug.get("n_layers", L)
    no_ar = debug.get("no_ar", False)
    no_gather = debug.get("no_gather", False)
    stop_f0 = debug.get("stop_f0", False)
    shared_ag = debug.get("shared_ag", False)
    N, SLICE, PAD, NT = cfg["N"], cfg["SLICE"], cfg["PAD"], cfg["NT"]
    WT, SPANS = cfg["WT"], cfg["SPANS"]
    cap, blk_start = sched["cap"], sched["blk_start"]
    call_start, call_len = sched["call_start"], sched["call_len"]
    S, NCH, first_w = sched["S"], sched["NCH"], sched["first_w"]
    inv_nd = 1.0 / (float(N) * float(D))
    tailz = PAD - SLICE
    max_ln = max(call_len)
    nspans = len(SPANS)

    nc = bacc.Bacc("TRN2", target_bir_lowering=False, debug=False,
                   enable_asserts=False, num_devices=NCORES,
                   num_swdge_queues=4)

    t_xT = nc.dram_tensor("xT", [2, 128, PAD], BF16, kind="ExternalInput")
    t_idx = nc.dram_tensor("idx", [128, S // 16], I16, kind="ExternalInput")
    t_b = nc.dram_tensor("bmat", [128, NCH, 128], BF16, kind="ExternalInput")
    t_l1w = nc.dram_tensor("lin1wT", [2, 128, 128], BF16, kind="ExternalInput")
    t_l1b = nc.dram_tensor("lin1b", [128, 1], F32, kind="ExternalInput")
    t_m1 = nc.dram_tensor("m1", [L, 128, 128], BF16, kind="ExternalInput")
    t_m2 = nc.dram_tensor("m2", [L, 128, 128], BF16, kind="ExternalInput")
    t_nw = nc.dram_tensor("nw", [128, 1], F32, kind="ExternalInput")
    t_nb = nc.dram_tensor("nb", [128, 1], F32, kind="ExternalInput")
    t_idb = nc.dram_tensor("identb", [128, 128], BF16, kind="ExternalInput")
    t_idf = nc.dram_tensor("identf", [128, 128], F32, kind="ExternalInput")
    t_y = nc.dram_tensor("y", [PAD, 128], F32, kind="ExternalOutput")

    rg = [list(range(NCORES))]
    ag_space = "Shared" if shared_ag else "Local"

    with tile.TileContext(nc) as tc:
        with tc.tile_pool(name="res", bufs=1) as res, \
             tc.tile_pool(name="gp", bufs=3) as gp, \
             tc.tile_pool(name="bb", bufs=2) as bbp, \
             tc.tile_pool(name="hb", bufs=4) as hbp, \
             tc.tile_pool(name="scr", bufs=2) as scrp, \
             tc.tile_pool(name="xt", bufs=2) as xtp, \
             tc.tile_pool(name="fn", bufs=2) as fnp, \
             tc.tile_pool(name="tr", bufs=2) as trp, \
             tc.tile_pool(name="sv", bufs=2) as sv, \
             tc.tile_pool(name="psA", bufs=1, space="PSUM") as psA, \
             tc.tile_pool(name="psB", bufs=2, space="PSUM") as psB, \
             tc.tile_pool(name="psT", bufs=1, space="PSUM") as psT, \
             tc.tile_pool(name="psM", bufs=1, space="PSUM") as psM, \
             tc.tile_pool(name="dram", bufs=1, space="DRAM") as dram:

            f_slice = [[dram.tile([(b - a) * 128, 128], BF16,
                                  name=f"f_slice_{li}_{w}")
                        for w, (a, b) in enumerate(WT)]
                       for li in range(L)]
            f_full = [[dram.tile([NCORES * (b - a) * 128, 128], BF16,
                                 name=f"f_full_{li}_{w}",
                                 addr_space=ag_space)
                       for w, (a, b) in enumerate(WT)]
                      for li in range(L)]
            ar_in = dram.tile([1, 8], F32)
            ar_out = dram.tile([1, 8], F32)

            # ---- resident loads
            idx_sb = res.tile([128, S // 16], I16)
            nc.sync.dma_start(idx_sb[:], t_idx[:])
            idb_sb = res.tile([128, 128], BF16)
            nc.sync.dma_start(idb_sb[:], t_idb[:])
            idf_sb = res.tile([128, 128], F32)
            nc.sync.dma_start(idf_sb[:], t_idf[:])
            l1w_sb = res.tile([128, 2, 128], BF16)
            nc.sync.dma_start(l1w_sb[:], t_l1w[:].rearrange("j k f -> k j f"))
            l1b_sb = res.tile([128, 1], F32)
            nc.sync.dma_start(l1b_sb[:], t_l1b[:])
            m1_sb = res.tile([128, L, 128], BF16)
            nc.sync.dma_start(m1_sb[:], t_m1[:].rearrange("l g f -> g l f"))
            m2_sb = res.tile([128, L, 128], BF16)
            nc.sync.dma_start(m2_sb[:], t_m2[:].rearrange("l g f -> g l f"))
            nw_sb = res.tile([128, 1], F32)
            nc.sync.dma_start(nw_sb[:], t_nw[:])
            nb_sb = res.tile([128, 1], F32)
            nc.sync.dma_start(nb_sb[:], t_nb[:])

            h_ps_all = psA.tile([128, 8, 128], F32, tag="H")
            x0_sb = res.tile([128, NT, 128], BF16)
            h_sb = res.tile([128, NT, 128], F32)   # H accum, then holds P
            acc_s = res.tile([128, NT], F32)
            acc_q = res.tile([128, NT], F32)
            ones_c = res.tile([128, 1], F32)
            nc.vector.memset(ones_c[:], 1.0)
            ones_r = res.tile([1, 128], F32)
            nc.vector.memset(ones_r[:], 1.0)

            def store_window(li, w, last):
                """normalize+relu tiles of window w from h_sb (holding P),
                transpose to node-major, store to f_slice[li+1][w] (or t_y)."""
                a, b = WT[w]
                for s0 in range(a, b, 4):
                    s1 = min(s0 + 4, b)
                    gsz = s1 - s0
                    dt_ = F32 if last else BF16
                    idm = idf_sb if last else idb_sb
                    fn = fnp.tile([128, 4, 128], dt_, tag=f"fn{dt_}")
                    nc.scalar.activation(fn[:, :gsz, :],
                                         h_sb[:, s0:s1, :],
                                         AF.Relu, bias=bv[:], scale=scv[:])
                    tr_ps = psT.tile([128, 4, 128], dt_, tag="Tf" if last else "T")
                    for j in range(gsz):
                        nc.tensor.transpose(tr_ps[:, j, :], fn[:, j, :],
                                            idm[:])
                    trs = trp.tile([128, 4, 128], dt_, tag=f"tr{dt_}")
                    nc.vector.tensor_copy(trs[:, :gsz, :], tr_ps[:, :gsz, :])
                    dst = t_y if last else f_slice[li + 1][w]
                    o0 = s0 * 128 if last else (s0 - a) * 128
                    nc.sync.dma_start(
                        dst[o0:o0 + gsz * 128, :].rearrange(
                            "(j d) f -> d j f", j=gsz),
                        trs[:, :gsz, :])

            # ---------------- phase 0: f0 = relu(lin1(x)), store, sub-AGs
            for w, (a, b) in enumerate(WT):
                for s0 in range(a, b, 4):
                    s1 = min(s0 + 4, b)
                    gsz = s1 - s0
                    xt = xtp.tile([128, 2, 4 * 128], BF16, tag="xt")
                    nc.sync.dma_start(
                        xt[:, :, :gsz * 128],
                        t_xT[:, :, s0 * 128:s1 * 128].rearrange(
                            "j k d -> k j d"))
                    f0_ps = psB.tile([128, 4, 128], F32, tag="P")
                    nc.tensor.matmul(f0_ps[:, :gsz, :], l1w_sb[:, 0, :],
                                     xt[:, 0, :gsz * 128], start=True,
                                     stop=False)
                    nc.tensor.matmul(f0_ps[:, :gsz, :], l1w_sb[:, 1, :],
                                     xt[:, 1, :gsz * 128], start=False,
                                     stop=True)
                    nc.scalar.activation(x0_sb[:, s0:s1, :],
                                         f0_ps[:, :gsz, :],
                                         AF.Relu, bias=l1b_sb[:], scale=1.0)
                    if s1 == NT and tailz > 0:
                        nc.vector.memset(x0_sb[:, NT - 1, 128 - tailz:], 0.0)
                    tr_ps = psT.tile([128, 4, 128], BF16, tag="T")
                    for j in range(gsz):
                        nc.tensor.transpose(tr_ps[:, j, :],
                                            x0_sb[:, s0 + j, :], idb_sb[:])
                    trs = trp.tile([128, 4, 128], BF16, tag=f"tr{BF16}")
                    nc.vector.tensor_copy(trs[:, :gsz, :], tr_ps[:, :gsz, :])
                    nc.sync.dma_start(
                        f_slice[0][w][(s0 - a) * 128:(s1 - a) * 128,
                                      :].rearrange(
                            "(j d) f -> d j f", j=gsz),
                        trs[:, :gsz, :])
                nc.gpsimd.collective_compute(
                    "AllGather", AL.bypass, replica_groups=rg,
                    ins=[f_slice[0][w].opt()], outs=[f_full[0][w].opt()])
            if stop_f0:
                for w, (a, b) in enumerate(WT):
                    nc.gpsimd.dma_start(
                        t_y[a * 128:b * 128, :],
                        f_full[0][w][:(b - a) * 128, :])  # core0 slice, cast
                n_layers_eff = 0
            else:
                n_layers_eff = n_layers

            # ---------------- layers
            hrot = [0]
            for li in range(n_layers_eff):
                last = li == L - 1
                for si, (w, s0, s1) in enumerate(SPANS):
                    a = call_start[si]
                    ln = call_len[si]
                    gt = gp.tile([128, max_ln // 128, 128], BF16, tag="g")
                    bts = bbp.tile([128, max_ln // 128, 128], BF16, tag="bb")
                    if ln:
                        nc.sync.dma_start(bts[:, :ln // 128, :],
                                          t_b[:, a // 128:(a + ln) // 128, :])
                        if no_gather:
                            nc.vector.memset(gt[:, :ln // 128, :], 0.0)
                        else:
                            nc.gpsimd.dma_gather(
                                gt[:, :ln // 128, :], f_full[li][w][:, :],
                                idx_sb[:, a // 16:(a + ln) // 16],
                                ln, ln, 128, single_packet=False,
                                queue_num=si % 4)
                    for t in range(s0, s1):
                        nck = int(cap[t, w]) // 128
                        if nck == 0:
                            continue
                        cl0 = (blk_start[(t, w)] - a) // 128
                        cg0 = blk_start[(t, w)] // 128
                        h_ps = h_ps_all[:, hrot[0] % 8, :]
                        hrot[0] += 1
                        cb0 = (blk_start[(t, w)] - a) // 128
                        for k in range(nck):
                            nc.tensor.matmul(
                                h_ps[:], gt[:, cl0 + k, :],
                                bts[:, cb0 + k, :],
                                start=(k == 0), stop=(k == nck - 1))
                        if w == first_w[t]:
                            nc.vector.tensor_copy(h_sb[:, t, :], h_ps[:])
                        else:
                            nc.vector.tensor_tensor(
                                h_sb[:, t, :], h_sb[:, t, :], h_ps[:],
                                op=AL.add)
                        if w == 3:
                            # H[t] complete: dense mixes + stats now
                            hbt = hbp.tile([128, 128], BF16, tag="hb")
                            nc.scalar.activation(hbt[:], h_sb[:, t, :],
                                                 AF.Copy)
                            p_ps = psB.tile([128, 4, 128], F32, tag="P")
                            nc.tensor.matmul(p_ps[:, 0, :], m1_sb[:, li, :],
                                             hbt[:], start=True, stop=False)
                            nc.tensor.matmul(p_ps[:, 0, :], m2_sb[:, li, :],
                                             x0_sb[:, t, :], start=False,
                                             stop=True)
                            nc.scalar.activation(
                                h_sb[:, t, :], p_ps[:, 0, :], AF.Copy,
                                accum_out=acc_s[:, t:t + 1])
                            scr = scrp.tile([128, 128], BF16, tag="scr")
                            nc.scalar.activation(scr[:], p_ps[:, 0, :],
                                                 AF.Square,
                                                 accum_out=acc_q[:, t:t + 1])

                # ---- global stats -> AllReduce -> scale/bias vectors
                tot = sv.tile([128, 2], F32, tag="tot")
                nc.vector.tensor_reduce(tot[:, 0:1], acc_s[:, :], axis=AX.X,
                                        op=AL.add)
                nc.vector.tensor_reduce(tot[:, 1:2], acc_q[:, :], axis=AX.X,
                                        op=AL.add)
                st_ps = psM.tile([128, 2], F32, tag="M")
                nc.tensor.matmul(st_ps[0:1, :], ones_c[:], tot[:],
                                 start=True, stop=True)
                st8 = sv.tile([1, 8], F32, tag="st8")
                nc.vector.memset(st8[:], 0.0)
                nc.vector.tensor_copy(st8[0:1, 0:2], st_ps[0:1, :])
                nc.sync.dma_start(ar_in[:], st8[:])
                if no_ar:
                    nc.sync.dma_start(ar_out[:], ar_in[:])
                else:
                    nc.gpsimd.collective_compute(
                        "AllReduce", AL.add, replica_groups=rg,
                        ins=[ar_in.opt()], outs=[ar_out.opt()])
                gs = sv.tile([1, 8], F32, tag="gs")
                nc.sync.dma_start(gs[:], ar_out[:])
                ms = sv.tile([1, 4], F32, tag="ms")
                nc.vector.tensor_scalar(ms[0:1, 0:1], gs[0:1, 0:1], inv_nd,
                                        None, op0=AL.mult)          # m
                nc.vector.tensor_scalar(ms[0:1, 1:2], gs[0:1, 1:2], inv_nd,
                                        None, op0=AL.mult)          # E[x^2]
                nc.vector.tensor_mul(ms[0:1, 2:3], ms[0:1, 0:1], ms[0:1, 0:1])
                nc.vector.tensor_sub(ms[0:1, 3:4], ms[0:1, 1:2], ms[0:1, 2:3])
                sq = sv.tile([1, 4], F32, tag="sq")
                nc.scalar.activation(sq[0:1, 0:1], ms[0:1, 3:4], AF.Sqrt)
                nc.vector.tensor_scalar(sq[0:1, 1:2], sq[0:1, 0:1], EPS, None,
                                        op0=AL.add)
                nc.vector.reciprocal(sq[0:1, 2:3], sq[0:1, 1:2])    # inv
                nc.vector.tensor_mul(sq[0:1, 3:4], sq[0:1, 2:3], ms[0:1, 0:1])
                pk = sv.tile([1, 2], F32, tag="pk")
                nc.vector.tensor_copy(pk[0:1, 0:1], sq[0:1, 2:3])
                nc.vector.tensor_copy(pk[0:1, 1:2], sq[0:1, 3:4])
                bc_ps = psM.tile([128, 2], F32, tag="M")
                nc.tensor.matmul(bc_ps[:], ones_r[:], pk[:],
                                 start=True, stop=True)
                bc = sv.tile([128, 2], F32, tag="bc")
                nc.vector.tensor_copy(bc[:], bc_ps[:])
                scv = sv.tile([128, 1], F32, tag="scv")
                nc.vector.tensor_mul(scv[:], bc[:, 0:1], nw_sb[:])
                bv1 = sv.tile([128, 1], F32, tag="bv1")
                nc.vector.tensor_mul(bv1[:], bc[:, 1:2], nw_sb[:])
                bv = sv.tile([128, 1], F32, tag="bv")
                nc.vector.tensor_sub(bv[:], nb_sb[:], bv1[:])

                # ---- normalize + relu + transpose + store + sub-AG
                for w in range(4):
                    store_window(li, w, last)
                    if not last:
                        nc.gpsimd.collective_compute(
                            "AllGather", AL.bypass, replica_groups=rg,
                            ins=[f_slice[li + 1][w].opt()],
                            outs=[f_full[li + 1][w].opt()])

    nc.compile()
    return nc


_last_results = None


def run(inputs, cfg, trace=False, debug=None):
    global _last_results
    sched, per_core, consts = preprocess(
        inputs["x"], inputs["edge_index"], inputs["lin1_w"], inputs["lin1_b"],
        inputs["w1"], inputs["w2"], inputs["norm_w"], inputs["norm_b"], cfg)
    nc = build(cfg, sched, debug=debug)
    in_maps = []
    for c in range(NCORES):
        m = dict(per_core[c])
        m.update(consts)
        in_maps.append(m)
    _last_results = run_bass_kernel_spmd(
        nc, in_maps, core_ids=list(range(NCORES)), trace=trace)
    SLICE = cfg["SLICE"]
    out = np.concatenate(
        [_last_results.results[c]["y"][:SLICE] for c in range(NCORES)], axis=0)
    return out.astype(np.float32)


def kernel(**inputs):
    return run(inputs, full_cfg(inputs["x"].shape[0]))


# revision 13
# speedup vs baseline: 1.4102x; 1.0833x over previous
"""GCNII backbone Bass/Trainium2 kernel — 8-core SPMD, v2.

Sharding: nodes row-partitioned across 8 cores (12500/core, padded to 12544
= 98 tiles of 128).  Edges live on the core that owns their *destination*
node.  Host-side index-only preprocessing builds a window-major,
destination-sorted, capacity-padded edge stream per core; the device does
everything float.

v2 structure (vs v1):
  * segment matrix B is built ON DEVICE per 128x128 chunk with one DVE
    dual-op tensor_scalar (is_equal by col, mult by enorm) from two small
    resident vectors — no 70MB/layer HBM stream of B.
  * f_full is laid out tile-major-interleaved and split into 4 window
    tensors f_full[w] (<=32768 rows each, int16-indexable).  Each window is
    AllGathered separately (Shared output) as soon as every core stored its
    slice piece, and the next layer's window-w gathers depend only on
    sub-AG w -> collectives pipeline behind compute.
  * window-major schedule: for w: gather spans of 14 dst tiles, chunk
    matmuls accumulate into PSUM per (tile,window), DVE folds into an SBUF
    f32 accumulator H; last window also feeds the dense P matmuls, stats,
    AllReduce, normalize, transpose, store, sub-AGs.
"""

import os
import sys

for _p in ("/opt/trn_rl_repo",):
    if _p not in sys.path:
        sys.path.insert(0, _p)

import math

import ml_dtypes
import numpy as np

import concourse.bacc as bacc
import concourse.bass as bass
import concourse.tile as tile
from concourse import mybir
from concourse.bass_utils import run_bass_kernel_spmd


def _register_en_onehot():
    """Custom fused DVE op: out[p,k] = (in0[p,k] == s0[p]) * s1[p].

    The stock dual-op tensor_scalar with two pointer scalars measures
    ~1.8us per [128,128] chunk on HW (scalars re-fetched per column); a
    fused custom-DVE op loads C0/C1 once and streams at full rate.
    """
    from concourse import dve_ops
    from concourse.dve_spec import Spec, Src0, C0, C1, eq, lower
    from concourse.dve_ops import has_src1
    from concourse.dve_uop import DveOpSpec
    from concourse.dve_table_gen import dve_ver_for

    name = "EN_ONEHOT"
    for op in dve_ops.OPS:
        if op.name == name:
            return op
    spec = Spec(
        body=eq(Src0, C0) * C1,
        reference=lambda in0, in1, s0, s1, imm2: ((in0 == s0) * s1),
    )
    row = dve_ops._CUSTOM_DVE_ROW_BASE + len(dve_ops.OPS)
    assert row < 0x20
    dve_ops._SUB_OPCODE_FOR_NAME[name] = row
    shas = {}
    for ver in ("v3", "v4"):
        try:
            dos = DveOpSpec(name=name, opcode=row, uops=lower(spec, ver=ver),
                            rd1_en=has_src1(spec))
            shas[ver] = dos.sha(ver)
        except Exception:
            pass
    op = dve_ops.DveOp(name, spec, subdim=False, uops_sha=shas)
    dve_ops.OPS.append(op)
    dve_ops.CUSTOM_DVE_SPECS[name] = spec
    return op


EN_ONEHOT = _register_en_onehot()

F32 = mybir.dt.float32
BF16 = mybir.dt.bfloat16
I16 = mybir.dt.int16
AX = mybir.AxisListType
AL = mybir.AluOpType
AF = mybir.ActivationFunctionType

NCORES = 8
D = 128
DIN = 256
L = 4
ALPHA = 0.5
THETA = 1.0
EPS = 1e-5


def _mk_cfg(N, slice_, sp):
    pad = ((slice_ + 127) // 128) * 128
    nt = pad // 128
    # split tiles into 4 windows (gather source ranges; one sub-AG each)
    base = -(-nt // 4)
    wt = []
    t0 = 0
    for w in range(4):
        t1 = min(t0 + base, nt)
        wt.append((t0, t1))
        t0 = t1
    assert wt[-1][1] == nt
    for (a, b) in wt:
        assert (b - a) * 128 * NCORES <= 32768  # int16-indexable window
    # spans: dst tiles per gather call; every src window sweeps ALL dst tiles
    spans = []
    for w in range(4):
        s0 = 0
        while s0 < nt:
            s1 = min(s0 + sp, nt)
            spans.append((w, s0, s1))
            s0 = s1
    return dict(N=N, SLICE=slice_, PAD=pad, NT=nt, WT=wt, SPANS=spans)


def full_cfg(N=100000):
    return _mk_cfg(N, N // NCORES, 3)


def small_cfg():
    return _mk_cfg(8000, 1000, 2)


# ---------------------------------------------------------------- host prep
def preprocess(x, edge_index, lin1_w, lin1_b, w1, w2, norm_w, norm_b, cfg):
    N, SLICE, PAD, NT = cfg["N"], cfg["SLICE"], cfg["PAD"], cfg["NT"]
    WT, SPANS = cfg["WT"], cfg["SPANS"]

    src = np.asarray(edge_index[0], dtype=np.int64)
    dst = np.asarray(edge_index[1], dtype=np.int64)
    sl = np.arange(N, dtype=np.int64)
    srcA = np.concatenate([src, sl])
    dstA = np.concatenate([dst, sl])

    deg = np.bincount(dstA, minlength=N).astype(np.float64)
    dis = 1.0 / np.sqrt(deg)
    en = ((1.0 - ALPHA) * dis[srcA] * dis[dstA]).astype(np.float32)

    # source address in its window tensor: f_full[w][c*TW*128 + (t-t0)*128 + r]
    s_core = srcA // SLICE
    s_loc = srcA % SLICE
    s_tile = s_loc // 128
    s_row = s_loc % 128
    t0s = np.zeros(NT, np.int64)
    wofs = np.zeros(NT, np.int64)  # window id per tile
    for w, (a, b) in enumerate(WT):
        wofs[a:b] = w
        t0s[a:b] = a
    twsize = np.array([b - a for (a, b) in WT], np.int64)
    s_w = wofs[s_tile]
    addr = (s_core * twsize[s_w] + (s_tile - t0s[s_tile])) * 128 + s_row
    assert addr.max() < 32768

    core = dstA // SLICE
    lt = (dstA % SLICE) // 128
    colr = ((dstA % SLICE) % 128).astype(np.float32)

    # per-(core, dst tile, window) counts -> shared static caps (mult of 128)
    blk = (core * NT + lt) * 4 + s_w
    cnt = np.bincount(blk, minlength=NCORES * NT * 4).reshape(NCORES, NT, 4)
    cap = (np.ceil(cnt.max(axis=0) / 128).astype(np.int64)) * 128  # [NT, 4]

    # stream block order: src-window-major, then dst tile
    border = [(t, w) for w in range(4) for t in range(NT)]
    blk_of = {tw: i for i, tw in enumerate(border)}
    blk_len = np.array([cap[t, w] for (t, w) in border], dtype=np.int64)
    blk_start_arr = np.concatenate([[0], np.cumsum(blk_len)])
    S_total = int(blk_start_arr[-1])
    NCH = S_total // 128
    blk_start = {tw: int(blk_start_arr[i]) for i, tw in enumerate(border)}

    # gather calls: one per span (w, s0, s1)
    call_start = [blk_start[(s0, w)] for (w, s0, s1) in SPANS]
    call_len = [int(cap[s0:s1, w].sum()) for (w, s0, s1) in SPANS]
    first_w = {}  # first window with edges, per tile
    for t in range(NT):
        first_w[t] = int(np.nonzero(cap[t, :])[0][0])

    sched = dict(cap=cap, blk_start=blk_start, call_start=call_start,
                 call_len=call_len, S=S_total, NCH=NCH, first_w=first_w)

    # per-core streams
    per_core = []
    bidx_all = np.array([blk_of[(int(t), int(w))] for t, w in zip(lt, s_w)],
                        dtype=np.int64)
    for c in range(NCORES):
        m = core == c
        bi = bidx_all[m]
        order = np.argsort(bi, kind="stable")
        bi_s = bi[order]
        cnts = np.bincount(bi_s, minlength=len(border))
        starts_sorted = np.concatenate([[0], np.cumsum(cnts)])[:-1]
        rank = np.arange(len(bi_s)) - starts_sorted[bi_s]
        pos = blk_start_arr[bi_s] + rank

        idx_s = np.zeros(S_total, np.int64)
        col_s = np.zeros(S_total, np.int64)
        en_s = np.zeros(S_total, np.float32)
        idx_s[pos] = addr[m][order]
        col_s[pos] = colr[m][order].astype(np.int64)
        en_s[pos] = en[m][order]

        # idx packed per gather call: wrap 16 partitions, replicate x8
        idxp = np.zeros((16, S_total // 16), np.int16)
        for a, ln in zip(call_start, call_len):
            if ln == 0:
                continue
            seg = idx_s[a:a + ln].astype(np.int16)
            idxp[:, a // 16:(a + ln) // 16] = seg.reshape(ln // 16, 16).T
        idxp = np.tile(idxp, (NCORES, 1))

        # host-built segment matrix B, streamed from HBM on device:
        # B[p, c, d] = enorm of edge (c*128+p) if its col == d else 0
        Bm = np.zeros((NCH, 128, 128), np.float32)
        Bm[np.arange(S_total) // 128, np.arange(S_total) % 128,
           col_s] = en_s
        Bm = np.ascontiguousarray(Bm.transpose(1, 0, 2)).astype(
            ml_dtypes.bfloat16)

        # x slice, transposed+packed on host: xT[j,k,d] = x[row d, 128j+k]
        xs = np.zeros((PAD, DIN), np.float32)
        xs[:SLICE] = np.asarray(x[c * SLICE:(c + 1) * SLICE], np.float32)
        xT = np.ascontiguousarray(
            xs.T.reshape(2, 128, PAD)).astype(ml_dtypes.bfloat16)

        per_core.append(dict(idx=idxp, bmat=Bm, xT=xT))

    # weights
    lw = np.asarray(lin1_w, np.float32)          # [128, 256]
    lin1wT = np.ascontiguousarray(lw.T.reshape(2, 128, 128)).astype(
        ml_dtypes.bfloat16)
    m1 = np.zeros((L, 128, 128), np.float32)
    m2 = np.zeros((L, 128, 128), np.float32)
    eye = np.eye(128, dtype=np.float32)
    for li in range(L):
        beta = float(np.log(THETA / (li + 1) + 1.0))
        m1[li] = (1.0 - beta) * eye + beta * np.asarray(w1[li], np.float32)
        m2[li] = ALPHA * ((1.0 - beta) * eye + beta * np.asarray(w2[li], np.float32))
    consts = dict(
        lin1wT=lin1wT,
        lin1b=np.asarray(lin1_b, np.float32).reshape(128, 1),
        m1=m1.astype(ml_dtypes.bfloat16), m2=m2.astype(ml_dtypes.bfloat16),
        nw=np.asarray(norm_w, np.float32).reshape(128, 1),
        nb=np.asarray(norm_b, np.float32).reshape(128, 1),
        identb=np.eye(128, dtype=np.float32).astype(ml_dtypes.bfloat16),
        identf=np.eye(128, dtype=np.float32),
    )
    return sched, per_core, consts


# ---------------------------------------------------------------- device IR
def build(cfg, sched, debug=None):
    debug = debug or {}
    n_layers = debug.get("n_layers", L)
    no_ar = debug.get("no_ar", False)
    no_gather = debug.get("no_gather", False)
    stop_f0 = debug.get("stop_f0", False)
    shared_ag = debug.get("shared_ag", False)
    N, SLICE, PAD, NT = cfg["N"], cfg["SLICE"], cfg["PAD"], cfg["NT"]
    WT, SPANS = cfg["WT"], cfg["SPANS"]
    cap, blk_start = sched["cap"], sched["blk_start"]
    call_start, call_len = sched["call_start"], sched["call_len"]
    S, NCH, first_w = sched["S"], sched["NCH"], sched["first_w"]
    inv_nd = 1.0 / (float(N) * float(D))
    tailz = PAD - SLICE
    max_ln = max(call_len)
    nspans = len(SPANS)

    nc = bacc.Bacc("TRN2", target_bir_lowering=False, debug=False,
                   enable_asserts=False, num_devices=NCORES,
                   num_swdge_queues=4)

    t_xT = nc.dram_tensor("xT", [2, 128, PAD], BF16, kind="ExternalInput")
    t_idx = nc.dram_tensor("idx", [128, S // 16], I16, kind="ExternalInput")
    t_b = nc.dram_tensor("bmat", [128, NCH, 128], BF16, kind="ExternalInput")
    t_l1w = nc.dram_tensor("lin1wT", [2, 128, 128], BF16, kind="ExternalInput")
    t_l1b = nc.dram_tensor("lin1b", [128, 1], F32, kind="ExternalInput")
    t_m1 = nc.dram_tensor("m1", [L, 128, 128], BF16, kind="ExternalInput")
    t_m2 = nc.dram_tensor("m2", [L, 128, 128], BF16, kind="ExternalInput")
    t_nw = nc.dram_tensor("nw", [128, 1], F32, kind="ExternalInput")
    t_nb = nc.dram_tensor("nb", [128, 1], F32, kind="ExternalInput")
    t_idb = nc.dram_tensor("identb", [128, 128], BF16, kind="ExternalInput")
    t_idf = nc.dram_tensor("identf", [128, 128], F32, kind="ExternalInput")
    t_y = nc.dram_tensor("y", [PAD, 128], F32, kind="ExternalOutput")

    rg = [list(range(NCORES))]
    ag_space = "Shared" if shared_ag else "Local"

    with tile.TileContext(nc) as tc:
        with tc.tile_pool(name="res", bufs=1) as res, \
             tc.tile_pool(name="gp", bufs=6) as gp, \
             tc.tile_pool(name="bb", bufs=4) as bbp, \
             tc.tile_pool(name="hb", bufs=4) as hbp, \
             tc.tile_pool(name="scr", bufs=2) as scrp, \
             tc.tile_pool(name="xt", bufs=2) as xtp, \
             tc.tile_pool(name="fn", bufs=2) as fnp, \
             tc.tile_pool(name="tr", bufs=2) as trp, \
             tc.tile_pool(name="sv", bufs=2) as sv, \
             tc.tile_pool(name="psA", bufs=1, space="PSUM") as psA, \
             tc.tile_pool(name="psB", bufs=2, space="PSUM") as psB, \
             tc.tile_pool(name="psT", bufs=1, space="PSUM") as psT, \
             tc.tile_pool(name="psM", bufs=1, space="PSUM") as psM, \
             tc.tile_pool(name="dram", bufs=1, space="DRAM") as dram:

            f_slice = [[dram.tile([(b - a) * 128, 128], BF16,
                                  name=f"f_slice_{li}_{w}")
                        for w, (a, b) in enumerate(WT)]
                       for li in range(L)]
            f_full = [[dram.tile([NCORES * (b - a) * 128, 128], BF16,
                                 name=f"f_full_{li}_{w}",
                                 addr_space=ag_space)
                       for w, (a, b) in enumerate(WT)]
                      for li in range(L)]
            ar_in = dram.tile([1, 8], F32)
            ar_out = dram.tile([1, 8], F32)

            # ---- resident loads
            idx_sb = res.tile([128, S // 16], I16)
            nc.sync.dma_start(idx_sb[:], t_idx[:])
            idb_sb = res.tile([128, 128], BF16)
            nc.sync.dma_start(idb_sb[:], t_idb[:])
            idf_sb = res.tile([128, 128], F32)
            nc.sync.dma_start(idf_sb[:], t_idf[:])
            l1w_sb = res.tile([128, 2, 128], BF16)
            nc.sync.dma_start(l1w_sb[:], t_l1w[:].rearrange("j k f -> k j f"))
            l1b_sb = res.tile([128, 1], F32)
            nc.sync.dma_start(l1b_sb[:], t_l1b[:])
            m1_sb = res.tile([128, L, 128], BF16)
            nc.sync.dma_start(m1_sb[:], t_m1[:].rearrange("l g f -> g l f"))
            m2_sb = res.tile([128, L, 128], BF16)
            nc.sync.dma_start(m2_sb[:], t_m2[:].rearrange("l g f -> g l f"))
            nw_sb = res.tile([128, 1], F32)
            nc.sync.dma_start(nw_sb[:], t_nw[:])
            nb_sb = res.tile([128, 1], F32)
            nc.sync.dma_start(nb_sb[:], t_nb[:])

            h_ps_all = psA.tile([128, 8, 128], F32, tag="H")
            x0_sb = res.tile([128, NT, 128], BF16)
            h_sb = res.tile([128, NT, 128], F32)   # H accum, then holds P
            acc_s = res.tile([128, NT], F32)
            acc_q = res.tile([128, NT], F32)
            ones_c = res.tile([128, 1], F32)
            nc.vector.memset(ones_c[:], 1.0)
            ones_r = res.tile([1, 128], F32)
            nc.vector.memset(ones_r[:], 1.0)

            def store_window(li, w, last):
                """normalize+relu tiles of window w from h_sb (holding P),
                transpose to node-major, store to f_slice[li+1][w] (or t_y)."""
                a, b = WT[w]
                for s0 in range(a, b, 4):
                    s1 = min(s0 + 4, b)
                    gsz = s1 - s0
                    dt_ = F32 if last else BF16
                    idm = idf_sb if last else idb_sb
                    fn = fnp.tile([128, 4, 128], dt_, tag=f"fn{dt_}")
                    nc.scalar.activation(fn[:, :gsz, :],
                                         h_sb[:, s0:s1, :],
                                         AF.Relu, bias=bv[:], scale=scv[:])
                    tr_ps = psT.tile([128, 4, 128], dt_, tag="Tf" if last else "T")
                    for j in range(gsz):
                        nc.tensor.transpose(tr_ps[:, j, :], fn[:, j, :],
                                            idm[:])
                    trs = trp.tile([128, 4, 128], dt_, tag=f"tr{dt_}")
                    nc.vector.tensor_copy(trs[:, :gsz, :], tr_ps[:, :gsz, :])
                    dst = t_y if last else f_slice[li + 1][w]
                    o0 = s0 * 128 if last else (s0 - a) * 128
                    nc.sync.dma_start(
                        dst[o0:o0 + gsz * 128, :].rearrange(
                            "(j d) f -> d j f", j=gsz),
                        trs[:, :gsz, :])

            # ---------------- phase 0: f0 = relu(lin1(x)), store, sub-AGs
            for w, (a, b) in enumerate(WT):
                for s0 in range(a, b, 4):
                    s1 = min(s0 + 4, b)
                    gsz = s1 - s0
                    xt = xtp.tile([128, 2, 4 * 128], BF16, tag="xt")
                    nc.sync.dma_start(
                        xt[:, :, :gsz * 128],
                        t_xT[:, :, s0 * 128:s1 * 128].rearrange(
                            "j k d -> k j d"))
                    f0_ps = psB.tile([128, 4, 128], F32, tag="P")
                    nc.tensor.matmul(f0_ps[:, :gsz, :], l1w_sb[:, 0, :],
                                     xt[:, 0, :gsz * 128], start=True,
                                     stop=False)
                    nc.tensor.matmul(f0_ps[:, :gsz, :], l1w_sb[:, 1, :],
                                     xt[:, 1, :gsz * 128], start=False,
                                     stop=True)
                    nc.scalar.activation(x0_sb[:, s0:s1, :],
                                         f0_ps[:, :gsz, :],
                                         AF.Relu, bias=l1b_sb[:], scale=1.0)
                    if s1 == NT and tailz > 0:
                        nc.vector.memset(x0_sb[:, NT - 1, 128 - tailz:], 0.0)
                    tr_ps = psT.tile([128, 4, 128], BF16, tag="T")
                    for j in range(gsz):
                        nc.tensor.transpose(tr_ps[:, j, :],
                                            x0_sb[:, s0 + j, :], idb_sb[:])
                    trs = trp.tile([128, 4, 128], BF16, tag=f"tr{BF16}")
                    nc.vector.tensor_copy(trs[:, :gsz, :], tr_ps[:, :gsz, :])
                    nc.sync.dma_start(
                        f_slice[0][w][(s0 - a) * 128:(s1 - a) * 128,
                                      :].rearrange(
                            "(j d) f -> d j f", j=gsz),
                        trs[:, :gsz, :])
                nc.gpsimd.collective_compute(
                    "AllGather", AL.bypass, replica_groups=rg,
                    ins=[f_slice[0][w].opt()], outs=[f_full[0][w].opt()])
            if stop_f0:
                for w, (a, b) in enumerate(WT):
                    nc.gpsimd.dma_start(
                        t_y[a * 128:b * 128, :],
                        f_full[0][w][:(b - a) * 128, :])  # core0 slice, cast
                n_layers_eff = 0
            else:
                n_layers_eff = n_layers

            # ---------------- layers
            hrot = [0]
            for li in range(n_layers_eff):
                last = li == L - 1
                for si, (w, s0, s1) in enumerate(SPANS):
                    a = call_start[si]
                    ln = call_len[si]
                    gt = gp.tile([128, max_ln // 128, 128], BF16, tag="g")
                    bts = bbp.tile([128, max_ln // 128, 128], BF16, tag="bb")
                    if ln:
                        nc.sync.dma_start(bts[:, :ln // 128, :],
                                          t_b[:, a // 128:(a + ln) // 128, :])
                        if no_gather:
                            nc.vector.memset(gt[:, :ln // 128, :], 0.0)
                        else:
                            nc.gpsimd.dma_gather(
                                gt[:, :ln // 128, :], f_full[li][w][:, :],
                                idx_sb[:, a // 16:(a + ln) // 16],
                                ln, ln, 128, single_packet=False,
                                queue_num=si % 4)
                    for t in range(s0, s1):
                        nck = int(cap[t, w]) // 128
                        if nck == 0:
                            continue
                        cl0 = (blk_start[(t, w)] - a) // 128
                        cg0 = blk_start[(t, w)] // 128
                        h_ps = h_ps_all[:, hrot[0] % 8, :]
                        hrot[0] += 1
                        cb0 = (blk_start[(t, w)] - a) // 128
                        for k in range(nck):
                            nc.tensor.matmul(
                                h_ps[:], gt[:, cl0 + k, :],
                                bts[:, cb0 + k, :],
                                start=(k == 0), stop=(k == nck - 1))
                        if w == first_w[t]:
                            nc.vector.tensor_copy(h_sb[:, t, :], h_ps[:])
                        else:
                            nc.vector.tensor_tensor(
                                h_sb[:, t, :], h_sb[:, t, :], h_ps[:],
                                op=AL.add)
                        if w == 3:
                            # H[t] complete: dense mixes + stats now
                            hbt = hbp.tile([128, 128], BF16, tag="hb")
                            nc.scalar.activation(hbt[:], h_sb[:, t, :],
                                                 AF.Copy)
                            p_ps = psB.tile([128, 4, 128], F32, tag="P")
                            nc.tensor.matmul(p_ps[:, 0, :], m1_sb[:, li, :],
                                             hbt[:], start=True, stop=False)
                            nc.tensor.matmul(p_ps[:, 0, :], m2_sb[:, li, :],
                                             x0_sb[:, t, :], start=False,
                                             stop=True)
                            nc.scalar.activation(
                                h_sb[:, t, :], p_ps[:, 0, :], AF.Copy,
                                accum_out=acc_s[:, t:t + 1])
                            scr = scrp.tile([128, 128], BF16, tag="scr")
                            nc.scalar.activation(scr[:], p_ps[:, 0, :],
                                                 AF.Square,
                                                 accum_out=acc_q[:, t:t + 1])

                # ---- global stats -> AllReduce -> scale/bias vectors
                tot = sv.tile([128, 2], F32, tag="tot")
                nc.vector.tensor_reduce(tot[:, 0:1], acc_s[:, :], axis=AX.X,
                                        op=AL.add)
                nc.vector.tensor_reduce(tot[:, 1:2], acc_q[:, :], axis=AX.X,
                                        op=AL.add)
                st_ps = psM.tile([128, 2], F32, tag="M")
                nc.tensor.matmul(st_ps[0:1, :], ones_c[:], tot[:],
                                 start=True, stop=True)
                st8 = sv.tile([1, 8], F32, tag="st8")
                nc.vector.memset(st8[:], 0.0)
                nc.vector.tensor_copy(st8[0:1, 0:2], st_ps[0:1, :])
                nc.sync.dma_start(ar_in[:], st8[:])
                if no_ar:
                    nc.sync.dma_start(ar_out[:], ar_in[:])
                else:
                    nc.gpsimd.collective_compute(
                        "AllReduce", AL.add, replica_groups=rg,
                        ins=[ar_in.opt()], outs=[ar_out.opt()])
                gs = sv.tile([1, 8], F32, tag="gs")
                nc.sync.dma_start(gs[:], ar_out[:])
                ms = sv.tile([1, 4], F32, tag="ms")
                nc.vector.tensor_scalar(ms[0:1, 0:1], gs[0:1, 0:1], inv_nd,
                                        None, op0=AL.mult)          # m
                nc.vector.tensor_scalar(ms[0:1, 1:2], gs[0:1, 1:2], inv_nd,
                                        None, op0=AL.mult)          # E[x^2]
                nc.vector.tensor_mul(ms[0:1, 2:3], ms[0:1, 0:1], ms[0:1, 0:1])
                nc.vector.tensor_sub(ms[0:1, 3:4], ms[0:1, 1:2], ms[0:1, 2:3])
                sq = sv.tile([1, 4], F32, tag="sq")
                nc.scalar.activation(sq[0:1, 0:1], ms[0:1, 3:4], AF.Sqrt)
                nc.vector.tensor_scalar(sq[0:1, 1:2], sq[0:1, 0:1], EPS, None,
                                        op0=AL.add)
                nc.vector.reciprocal(sq[0:1, 2:3], sq[0:1, 1:2])    # inv
                nc.vector.tensor_mul(sq[0:1, 3:4], sq[0:1, 2:3], ms[0:1, 0:1])
                pk = sv.tile([1, 2], F32, tag="pk")
                nc.vector.tensor_copy(pk[0:1, 0:1], sq[0:1, 2:3])
                nc.vector.tensor_copy(pk[0:1, 1:2], sq[0:1, 3:4])
                bc_ps = psM.tile([128, 2], F32, tag="M")
                nc.tensor.matmul(bc_ps[:], ones_r[:], pk[:],
                                 start=True, stop=True)
                bc = sv.tile([128, 2], F32, tag="bc")
                nc.vector.tensor_copy(bc[:], bc_ps[:])
                scv = sv.tile([128, 1], F32, tag="scv")
                nc.vector.tensor_mul(scv[:], bc[:, 0:1], nw_sb[:])
                bv1 = sv.tile([128, 1], F32, tag="bv1")
                nc.vector.tensor_mul(bv1[:], bc[:, 1:2], nw_sb[:])
                bv = sv.tile([128, 1], F32, tag="bv")
                nc.vector.tensor_sub(bv[:], nb_sb[:], bv1[:])

                # ---- normalize + relu + transpose + store + sub-AG
                for w in range(4):
                    store_window(li, w, last)
                    if not last:
                        nc.gpsimd.collective_compute(
                            "AllGather", AL.bypass, replica_groups=rg,
                            ins=[f_slice[li + 1][w].opt()],
                            outs=[f_full[li + 1][w].opt()])

    nc.compile()
    return nc


_last_results = None


def run(inputs, cfg, trace=False, debug=None):
    global _last_results
    sched, per_core, consts = preprocess(
        inputs["x"], inputs["edge_index"], inputs["lin1_w"], inputs["lin1_b"],
        inputs["w1"], inputs["w2"], inputs["norm_w"], inputs["norm_b"], cfg)
    nc = build(cfg, sched, debug=debug)
    in_maps = []
    for c in range(NCORES):
        m = dict(per_core[c])
        m.update(consts)
        in_maps.append(m)
    _last_results = run_bass_kernel_spmd(
        nc, in_maps, core_ids=list(range(NCORES)), trace=trace)
    SLICE = cfg["SLICE"]
    out = np.concatenate(
        [_last_results.results[c]["y"][:SLICE] for c in range(NCORES)], axis=0)
    return out.astype(np.float32)


def kernel(**inputs):
    return run(inputs, full_cfg(inputs["x"].shape[0]))
